# revision 49
# baseline (speedup 1.0000x reference)
# DeepSet Trainium2 kernel.
#
# Strategy: events are sorted by jet-count n (2..10) on the host and
# round-robin sharded across 8 cores into per-group slots of capacity cap_g
# (multiple of 8, exact-packed). Within a group every event has exactly n=g
# valid jets, so all masks, pair structures and aggregation counts are
# compile-time constants.
#
# Math folding (host, O(params)):
#   every Dense+BN+relu block becomes relu(h @ W' + b') with W', b' folded.
#   MLP2 layer 1 uses the z-trick: y1 = relu(z_i + z_j + t) with z = x @ Wz'.
#   t is folded into the y1 relu pass (tensor_scalar add-bias+max0).
#
# Device layout: feature-major [H=128 partitions, columns = slice*cap + b]
# per group, all activations bf16 (PE: 1 col/cycle), PSUM f32.
# Work distribution across engines:
#   PE:   all matmuls + Sum/Sumsq of pairs via PSUM-accumulating identity mms
#   Act:  x1/x2/z/x PSUM evacs (relu+bias / copy), y2 evac, xsq (Square)
#   DVE:  x-side sum/sumsq/max trees, y1 add (broadcast AP) + relu,
#         ysq (y3*y3), y-side max tree
#   Pool: y3 PSUM evac (tensor_scalar bias+relu)
# Mean/Var and the final [events, 4H] transpose are computed on the HOST
# from the 6 DMA'd feature-major aggregates (sum/sumsq/max per side).
import math
from contextlib import ExitStack

import numpy as np

import concourse.bass as bass
import concourse.bacc as bacc
import concourse.tile as tile
import concourse.mybir as mybir

f32 = mybir.dt.float32
bf16 = mybir.dt.bfloat16
AF = mybir.ActivationFunctionType
ALU = mybir.AluOpType

H = 128
FJ = 16


def pairs_of(g):
    return [(i, j) for i in range(g) for j in range(i + 1, g)]


# GPSIMD findings (measured): cannot access PSUM (birverifier), has no
# max opcode, tensor_scalar runs ~12ns/col, and even its decent-rate
# tensor_tensor mult slows the Vector engine ~20-30% via SBUF port
# contention. Net negative everywhere -> unused.
# pow is rejected by the DVE ISA check (tensor_scalar_valid_ops).
USE_POW_SQUARE = False


def build_program(groups):
    """groups: list of (g, cap) with cap a multiple of 8, cap <= 512."""
    JC = sum(g * cap for g, cap in groups)
    EC = sum(cap for _, cap in groups)

    nc = bacc.Bacc("TRN2", target_bir_lowering=False, debug=False)

    jets_d = nc.dram_tensor("jets", [FJ, JC], bf16, kind="ExternalInput")
    w1_d = nc.dram_tensor("w1", [FJ, H], bf16, kind="ExternalInput")
    w2_d = nc.dram_tensor("w2", [H, H], bf16, kind="ExternalInput")
    w3_d = nc.dram_tensor("w3", [H, H], bf16, kind="ExternalInput")
    wz_d = nc.dram_tensor("wz", [H, H], bf16, kind="ExternalInput")
    w4_d = nc.dram_tensor("w4", [H, H], bf16, kind="ExternalInput")
    w5_d = nc.dram_tensor("w5", [H, H], bf16, kind="ExternalInput")
    identp_d = nc.dram_tensor("identp", [H, H], bf16, kind="ExternalInput")
    # bias vector cols: 0..5 = b1, b2, b3, t(=bz), b4, b5
    bv_d = nc.dram_tensor("bvec", [H, 8], f32, kind="ExternalInput")
    # per group: 6 aggregates [H, cap] each, packed [sx qx mx sy qy my]
    out6_d = nc.dram_tensor("out6", [H, 6 * EC], f32, kind="ExternalOutput")

    with tile.TileContext(nc) as tc, ExitStack() as ctx:
        consts = ctx.enter_context(tc.tile_pool(name="consts", bufs=1))
        jin = ctx.enter_context(tc.tile_pool(name="jin", bufs=2))
        x12 = ctx.enter_context(tc.tile_pool(name="x12", bufs=2))
        bigx = ctx.enter_context(tc.tile_pool(name="bigx", bufs=2))
        bigy = ctx.enter_context(tc.tile_pool(name="bigy", bufs=2))
        scr = ctx.enter_context(tc.tile_pool(name="scr", bufs=2))
        mxp = ctx.enter_context(tc.tile_pool(name="mxp", bufs=2))
        aggs = ctx.enter_context(tc.tile_pool(name="aggs", bufs=2))
        mm = ctx.enter_context(tc.tile_pool(name="mm", bufs=2, space="PSUM"))
        acc = ctx.enter_context(tc.tile_pool(name="acc", bufs=1, space="PSUM"))

        def const_tile(name, dram, shape, dt):
            t = consts.tile(shape, dt, tag=name)
            nc.sync.dma_start(t[:], dram.ap())
            return t

        w1t = const_tile("w1", w1_d, [FJ, H], bf16)
        w2t = const_tile("w2", w2_d, [H, H], bf16)
        w3t = const_tile("w3", w3_d, [H, H], bf16)
        wzt = const_tile("wz", wz_d, [H, H], bf16)
        w4t = const_tile("w4", w4_d, [H, H], bf16)
        w5t = const_tile("w5", w5_d, [H, H], bf16)
        ip_t = const_tile("ip", identp_d, [H, H], bf16)
        bv = const_tile("bv", bv_d, [H, 8], f32)

        def r3(ap, k):
            return ap.rearrange("p (k c) -> p k c", k=k)

        # square via DVE tensor_scalar pow: hits the 4x bf16 perf mode
        # (tensor_tensor mult only reaches ~1 elem/cycle). Inputs are
        # relu outputs (>= 0) so pow is safe.
        def square(dst, src):
            if USE_POW_SQUARE:
                nc.vector.tensor_scalar(dst, src, 2.0, None, ALU.pow)
            else:
                nc.vector.tensor_tensor(dst, src, src, ALU.mult)

        # ---- jets side: 4 layers as a list of emission steps (one
        # 1024-col layer-chunk each). Steps of group g+1 are interleaved
        # into group g's pair-chunk loop: PE and Scalar advance the next
        # group's jets while Vector drains this group's pair work, so
        # Vector rolls straight into the next y1 with z already ready.
        def make_jets(g, cap, jets_off):
            JCg = g * cap
            jt = jin.tile([FJ, JCg], bf16, tag="jt")
            x1 = x12.tile([H, JCg], bf16, tag="x1")
            x2 = x12.tile([H, JCg], bf16, tag="x2")
            x = bigx.tile([H, JCg], bf16, tag="x")
            z = bigx.tile([H, JCg], bf16, tag="z")

            def dma_step(jt=jt, jets_off=jets_off, JCg=JCg):
                nc.sync.dma_start(jt[:],
                                  jets_d.ap()[:, jets_off : jets_off + JCg])

            steps = [dma_step]
            plan = [(x1, w1t, jt, 0), (x2, w2t, x1, 1),
                    (x, w3t, x2, 2), (z, wzt, x, None)]
            for dst, wt, src, bias_col in plan:
                for c0 in range(0, JCg, 1024):
                    w = min(1024, JCg - c0)

                    def step(dst=dst, wt=wt, src=src, bias_col=bias_col,
                             c0=c0, w=w):
                        ps = mm.tile([H, 1024], f32, tag="mm")
                        for s0 in range(0, w, 512):
                            sw = min(512, w - s0)
                            nc.tensor.matmul(ps[:, s0 : s0 + sw], wt[:],
                                             src[:, c0 + s0 : c0 + s0 + sw],
                                             start=True, stop=True)
                        if bias_col is None:
                            nc.scalar.copy(dst[:, c0 : c0 + w], ps[:, :w])
                        else:
                            nc.scalar.activation(
                                dst[:, c0 : c0 + w], ps[:, :w], AF.Relu,
                                bias=bv[:, bias_col : bias_col + 1])

                    steps.append(step)
            return steps, x, z

        ev_off = 0
        pending_tail = [None]
        g0, cap0 = groups[0]
        cur_steps, cur_x, cur_z = make_jets(g0, cap0, 0)
        for st in cur_steps:
            st()
        jets_off = g0 * cap0
        for gi, (g, cap) in enumerate(groups):
            assert cap % 8 == 0 and cap <= 512
            JCg = g * cap
            prs = pairs_of(g)
            PG = len(prs)
            x, z = cur_x, cur_z

            if gi + 1 < len(groups):
                g2, cap2 = groups[gi + 1]
                nxt_steps, nxt_x, nxt_z = make_jets(g2, cap2, jets_off)
                jets_off += g2 * cap2
            else:
                nxt_steps, nxt_x, nxt_z = [], None, None
            squeue = list(nxt_steps)

            agg6 = aggs.tile([H, 6 * cap], f32, tag="agg6")

            def max_tree(src_tile, nslices, dst_f32, tag, l1_pool=False,
                         cap=cap):
                m, cur, off = nslices, src_tile, 0
                if m == 1:
                    nc.vector.tensor_copy(dst_f32, src_tile[:, 0:cap])
                    return
                first = True
                while m > 1:
                    k2 = (m + 1) // 2
                    if k2 == 1:
                        nxt = dst_f32
                    else:
                        nxt = mxp.tile([H, k2 * cap], bf16, tag=tag)
                    a0 = cur[:, off : off + k2 * cap]
                    a1 = cur[:, off + (m - k2) * cap : off + m * cap]
                    nc.vector.tensor_tensor(r3(nxt[:, 0 : k2 * cap], k2),
                                            r3(a0, k2), r3(a1, k2), ALU.max)
                    cur, off, m = nxt, 0, k2
                    first = False

            # ---- pairs side: SC-slice chunks, lag-1 ysq+sums.
            SC = max(1, 2048 // cap)
            PCH = SC * cap
            y3 = bigy.tile([H, PG * cap], bf16, tag="y3")
            a_sy = acc.tile([H, cap], f32, tag="a_sy")
            a_qy = acc.tile([H, cap], f32, tag="a_qy")
            a_sx = acc.tile([H, cap], f32, tag="a_sx")
            a_qx = acc.tile([H, cap], f32, tag="a_qx")

            def ysq_sums(p0, k, cap=cap, PCH=PCH, y3=y3, PG=PG,
                         a_sy=a_sy, a_qy=a_qy):
                w = k * cap
                ysq = scr.tile([H, PCH], bf16, tag="ysq")
                square(ysq[:, :w], y3[:, p0 * cap : p0 * cap + w])
                for s in range(k):
                    sl = p0 + s
                    nc.tensor.matmul(a_sy[:, :cap], ip_t[:],
                                     y3[:, sl * cap : (sl + 1) * cap],
                                     start=(sl == 0), stop=(sl == PG - 1))
                for s in range(k):
                    sl = p0 + s
                    nc.tensor.matmul(a_qy[:, :cap], ip_t[:],
                                     ysq[:, s * cap : (s + 1) * cap],
                                     start=(sl == 0), stop=(sl == PG - 1))

            prev = None
            y3_i = [0]
            chunk_starts = list(range(0, PG, SC))
            for ci, p0 in enumerate(chunk_starts):
                k = min(SC, PG - p0)
                w = k * cap
                y1 = scr.tile([H, PCH], bf16, tag="y1")
                s = 0
                while s < k:
                    i = prs[p0 + s][0]
                    r = 1
                    while s + r < k and prs[p0 + s + r][0] == i:
                        r += 1
                    j0 = prs[p0 + s][1]
                    nc.vector.tensor_tensor(
                        r3(y1[:, s * cap : (s + r) * cap], r),
                        r3(z[:, i * cap : (i + 1) * cap], 1).broadcast_to(
                            [H, r, cap]),
                        r3(z[:, j0 * cap : (j0 + r) * cap], r),
                        ALU.add)
                    s += r
                nc.vector.tensor_scalar(y1[:, :w], y1[:, :w], bv[:, 3:4],
                                        0.0, ALU.add, ALU.max)
                y2 = scr.tile([H, PCH], bf16, tag="y2")
                for n0 in range(0, w, 1024):
                    cw = min(1024, w - n0)
                    ps = mm.tile([H, 1024], f32, tag="mm")
                    for s0 in range(0, cw, 512):
                        sw = min(512, cw - s0)
                        nc.tensor.matmul(ps[:, s0 : s0 + sw], w4t[:],
                                         y1[:, n0 + s0 : n0 + s0 + sw],
                                         start=True, stop=True)
                    nc.scalar.activation(y2[:, n0 : n0 + cw], ps[:, :cw],
                                         AF.Relu, bias=bv[:, 4:5])
                for n0 in range(0, w, 1024):
                    cw = min(1024, w - n0)
                    ps = mm.tile([H, 1024], f32, tag="mm")
                    for s0 in range(0, cw, 512):
                        sw = min(512, cw - s0)
                        nc.tensor.matmul(ps[:, s0 : s0 + sw], w5t[:],
                                         y2[:, n0 + s0 : n0 + s0 + sw],
                                         start=True, stop=True)
                    dst = y3[:, p0 * cap + n0 : p0 * cap + n0 + cw]
                    if y3_i[0] % 2 == 0:
                        nc.scalar.activation(dst, ps[:, :cw], AF.Relu,
                                             bias=bv[:, 5:6])
                    else:
                        nc.vector.tensor_scalar(dst, ps[:, :cw], bv[:, 5:6],
                                                0.0, ALU.add, ALU.max)
                    y3_i[0] += 1
                if prev is not None:
                    ysq_sums(*prev)
                prev = (p0, k)
                # Pace the next group's jets steps across this group's
                # remaining chunks (all leftovers drain after the loop).
                rem = len(chunk_starts) - 1 - ci
                if rem > 0:
                    npop = -(-len(squeue) // (rem + 1))
                    for _ in range(min(npop, len(squeue))):
                        squeue.pop(0)()
                # The previous group's tail goes one chunk deep into this
                # group: V starts this group's y1 immediately (unblocking
                # PE and Scalar), and the tail fills V/PE slack while the
                # first chunk's matmuls run.
                if pending_tail[0] is not None:
                    pending_tail[0]()
                    pending_tail[0] = None

            for st in squeue:
                st()

            # x-side square/max emitted after the pair chunks: the Vector
            # engine runs them while PE finishes the pair matmuls, instead
            # of delaying the first y1 (which PE waits on).
            xsq = bigx.tile([H, JCg], bf16, tag="xsq")
            square(xsq[:], x[:])
            max_tree(x, g, agg6[:, 2 * cap : 3 * cap], "xt")

            def tail(prev=prev, y3=y3, PG=PG, g=g, cap=cap, agg6=agg6,
                     a_sy=a_sy, a_qy=a_qy, a_sx=a_sx, a_qx=a_qx, xsq=xsq,
                     x=x, ev_off=ev_off, max_tree=max_tree,
                     ysq_sums=ysq_sums):
                ysq_sums(*prev)
                for s in range(g):
                    nc.tensor.matmul(a_sx[:, :cap], ip_t[:],
                                     x[:, s * cap : (s + 1) * cap],
                                     start=(s == 0), stop=(s == g - 1))
                for s in range(g):
                    nc.tensor.matmul(a_qx[:, :cap], ip_t[:],
                                     xsq[:, s * cap : (s + 1) * cap],
                                     start=(s == 0), stop=(s == g - 1))
                max_tree(y3, PG, agg6[:, 5 * cap : 6 * cap], "yt",
                         l1_pool=True)
                nc.scalar.copy(agg6[:, 0:cap], a_sx[:, :cap])
                nc.scalar.copy(agg6[:, cap : 2 * cap], a_qx[:, :cap])
                nc.scalar.copy(agg6[:, 3 * cap : 4 * cap], a_sy[:, :cap])
                nc.scalar.copy(agg6[:, 4 * cap : 5 * cap], a_qy[:, :cap])
                nc.sync.dma_start(
                    out6_d.ap()[:, 6 * ev_off : 6 * ev_off + 6 * cap],
                    agg6[:])

            pending_tail[0] = tail

            cur_steps, cur_x, cur_z = nxt_steps, nxt_x, nxt_z
            ev_off += cap
        pending_tail[0]()

    nc.compile()
    return nc


# ---------------- host-side math ----------------

BN_EPS = 1e-3


def fold_params(inp):
    """Fold normalization + BN into per-layer (W, b). All numpy fp32."""
    mean_j = np.asarray(inp["mean_jets"], np.float32)
    std_j = np.asarray(inp["std_jets"], np.float32)
    w1f = np.asarray(inp["w1_first"], np.float32)
    w1r = np.asarray(inp["w1_rest"], np.float32)
    bn1 = np.asarray(inp["bn1"], np.float32)  # [3,4,H]: gamma, beta, mean, var
    w2f = np.asarray(inp["w2_first"], np.float32)
    w2r = np.asarray(inp["w2_rest"], np.float32)
    bn2 = np.asarray(inp["bn2"], np.float32)

    def bn_sb(row):
        gm, bt, mu, vv = row[0], row[1], row[2], row[3]
        s = gm / np.sqrt(vv + BN_EPS)
        return s.astype(np.float32), (bt - mu * s).astype(np.float32)

    s11, t11 = bn_sb(bn1[0]); s12, t12 = bn_sb(bn1[1]); s13, t13 = bn_sb(bn1[2])
    s21, t21 = bn_sb(bn2[0]); s22, t22 = bn_sb(bn2[1]); s23, t23 = bn_sb(bn2[2])

    A = w1f / std_j[:, None]
    c = -(mean_j / std_j) @ w1f
    return dict(
        W1=A * s11[None, :], b1=c * s11 + t11,
        W2=w1r[0] * s12[None, :], b2=t12,
        W3=w1r[1] * s13[None, :], b3=t13,
        Wz=w2f * s21[None, :], bz=t21,
        W4=w2r[0] * s22[None, :], b4=t22,
        W5=w2r[1] * s23[None, :], b5=t23,
    )


# ---------------- full kernel entry point ----------------

N_CORES = 8

_cache = {}
_TRACE = [False]
_LAST_RESULT = [None]


def _get_program(groups_key):
    if groups_key not in _cache:
        _cache[groups_key] = build_program(list(groups_key))
    return _cache[groups_key]


def _np_dt(dt):
    return mybir.dt.np(dt)


def _plan(n):
    """Returns (groups, slots): groups = [(g, cap)], slots[c][gi] =
    (padded index array, real count) for core c, group gi."""
    gs = []
    idx_by_g = {}
    for g in range(2, 11):
        idx = np.nonzero(n == g)[0]
        if len(idx):
            gs.append(g)
            idx_by_g[g] = idx
    stray = np.nonzero((n < 2) | (n > 10))[0]
    if len(stray):
        if not gs:
            gs.append(2)
            idx_by_g[2] = stray
        else:
            idx_by_g[gs[-1]] = np.concatenate([idx_by_g[gs[-1]], stray])
    # Descending size order: each group's jets chain is emitted inside
    # the previous (bigger) group's pair phase, so it pipelines fully.
    gs = sorted(gs, key=lambda g: -g)
    groups = []
    slots = [[] for _ in range(N_CORES)]
    for g in gs:
        idx = idx_by_g[g]
        per_core = [idx[c::N_CORES] for c in range(N_CORES)]
        mx = max(len(p) for p in per_core)
        cap = max(8, ((mx + 7) // 8) * 8)
        groups.append((g, cap))
        fill = idx[0]
        for c in range(N_CORES):
            p = per_core[c]
            pad = np.full(cap, p[0] if len(p) else fill, dtype=np.int64)
            pad[: len(p)] = p
            slots[c].append((pad, len(p)))
    return groups, slots


def _pack_jets(jets, groups, slots_c):
    cols = []
    for (g, cap), (ids, _cnt) in zip(groups, slots_c):
        ev = jets[ids][:, :g, :]  # [cap, g, 16]
        cols.append(np.ascontiguousarray(ev.transpose(2, 1, 0)).reshape(
            FJ, g * cap))
    return np.concatenate(cols, axis=1).astype(_np_dt(bf16), copy=False)


def kernel(**inputs):
    from concourse.bass_utils import run_bass_kernel_spmd

    jets = np.asarray(inputs["inputs_jets"], dtype=np.float32)
    B = jets.shape[0]
    mask = (jets != 0.0).any(-1)
    n = mask.sum(-1).astype(np.int64)
    # compact valid jets to the front (no-op for the standard generator)
    if not np.array_equal(mask, np.arange(jets.shape[1])[None, :] < n[:, None]):
        order = np.argsort(~mask, axis=1, kind="stable")
        jets = np.take_along_axis(jets, order[:, :, None], axis=1)

    P = fold_params(inputs)
    groups, slots = _plan(n)
    nc = _get_program(tuple(groups))

    bvec = np.zeros((H, 8), np.float32)
    for i, k in enumerate(["b1", "b2", "b3", "bz", "b4", "b5"]):
        bvec[:, i] = P[k]
    ident = np.eye(H, dtype=np.float32)
    bnp = _np_dt(bf16)
    common = {
        "w1": P["W1"].astype(bnp), "w2": P["W2"].astype(bnp),
        "w3": P["W3"].astype(bnp), "wz": P["Wz"].astype(bnp),
        "w4": P["W4"].astype(bnp), "w5": P["W5"].astype(bnp),
        "identp": ident.astype(bnp), "bvec": bvec,
    }
    in_maps = []
    for c in range(N_CORES):
        m = dict(common)
        m["jets"] = _pack_jets(jets, groups, slots[c])
        in_maps.append(m)

    res = run_bass_kernel_spmd(nc, in_maps, core_ids=list(range(N_CORES)),
                               trace=_TRACE[0])
    _LAST_RESULT[0] = res

    agg_x = np.empty((B, 4 * H), np.float32)
    agg_y = np.empty((B, 4 * H), np.float32)
    for c in range(N_CORES):
        o6 = res.results[c]["out6"]  # [H, 6*EC] f32
        ev_off = 0
        for (g, cap), (ids, cnt) in zip(groups, slots[c]):
            blk = o6[:, 6 * ev_off : 6 * ev_off + 6 * cap]
            sx = blk[:, 0:cap].T[:cnt]
            qx = blk[:, cap : 2 * cap].T[:cnt]
            mx = blk[:, 2 * cap : 3 * cap].T[:cnt]
            sy = blk[:, 3 * cap : 4 * cap].T[:cnt]
            qy = blk[:, 4 * cap : 5 * cap].T[:cnt]
            my = blk[:, 5 * cap : 6 * cap].T[:cnt]
            ii = ids[:cnt]
            mean_x = sx / g
            mean_y = sy / (g * (g - 1) // 2)
            agg_x[ii] = np.concatenate(
                [sx, mx, mean_x, qx / g - mean_x * mean_x], axis=1)
            agg_y[ii] = np.concatenate(
                [sy, my, mean_y, qy / (g * (g - 1) // 2) - mean_y * mean_y],
                axis=1)
            ev_off += cap
    return agg_x, agg_y


# revision 53
# speedup vs baseline: 1.0017x; 1.0017x over previous
# DeepSet Trainium2 kernel.
#
# Strategy: events are sorted by jet-count n (2..10) on the host and
# round-robin sharded across 8 cores into per-group slots of capacity cap_g
# (multiple of 8, exact-packed). Within a group every event has exactly n=g
# valid jets, so all masks, pair structures and aggregation counts are
# compile-time constants.
#
# Math folding (host, O(params)):
#   every Dense+BN+relu block becomes relu(h @ W' + b') with W', b' folded.
#   MLP2 layer 1 uses the z-trick: y1 = relu(z_i + z_j + t) with z = x @ Wz'.
#   t is folded into the y1 relu pass (tensor_scalar add-bias+max0).
#
# Device layout: feature-major [H=128 partitions, columns = slice*cap + b]
# per group, all activations bf16 (PE: 1 col/cycle), PSUM f32.
# Work distribution across engines:
#   PE:   all matmuls + Sum/Sumsq of pairs via PSUM-accumulating identity mms
#   Act:  x1/x2/z/x PSUM evacs (relu+bias / copy), y2 evac, xsq (Square)
#   DVE:  x-side sum/sumsq/max trees, y1 add (broadcast AP) + relu,
#         ysq (y3*y3), y-side max tree
#   Pool: y3 PSUM evac (tensor_scalar bias+relu)
# Mean/Var and the final [events, 4H] transpose are computed on the HOST
# from the 6 DMA'd feature-major aggregates (sum/sumsq/max per side).
import math
from contextlib import ExitStack

import numpy as np

import concourse.bass as bass
import concourse.bacc as bacc
import concourse.tile as tile
import concourse.mybir as mybir

f32 = mybir.dt.float32
bf16 = mybir.dt.bfloat16
AF = mybir.ActivationFunctionType
ALU = mybir.AluOpType

H = 128
FJ = 16


def pairs_of(g):
    return [(i, j) for i in range(g) for j in range(i + 1, g)]


# GPSIMD findings (measured): cannot access PSUM (birverifier), has no
# max opcode, tensor_scalar runs ~12ns/col, and even its decent-rate
# tensor_tensor mult slows the Vector engine ~20-30% via SBUF port
# contention. Net negative everywhere -> unused.
# pow is rejected by the DVE ISA check (tensor_scalar_valid_ops).
USE_POW_SQUARE = False


def build_program(groups):
    """groups: list of (g, cap) with cap a multiple of 8, cap <= 512."""
    JC = sum(g * cap for g, cap in groups)
    EC = sum(cap for _, cap in groups)

    nc = bacc.Bacc("TRN2", target_bir_lowering=False, debug=False)

    jets_d = nc.dram_tensor("jets", [FJ, JC], bf16, kind="ExternalInput")
    w1_d = nc.dram_tensor("w1", [FJ, H], bf16, kind="ExternalInput")
    w2_d = nc.dram_tensor("w2", [H, H], bf16, kind="ExternalInput")
    w3_d = nc.dram_tensor("w3", [H, H], bf16, kind="ExternalInput")
    wz_d = nc.dram_tensor("wz", [H, H], bf16, kind="ExternalInput")
    w4_d = nc.dram_tensor("w4", [H, H], bf16, kind="ExternalInput")
    w5_d = nc.dram_tensor("w5", [H, H], bf16, kind="ExternalInput")
    identp_d = nc.dram_tensor("identp", [H, H], bf16, kind="ExternalInput")
    # bias vector cols: 0..5 = b1, b2, b3, t(=bz), b4, b5
    bv_d = nc.dram_tensor("bvec", [H, 8], f32, kind="ExternalInput")
    # per group: 6 aggregates [H, cap] each, packed [sx qx mx sy qy my]
    out6_d = nc.dram_tensor("out6", [H, 6 * EC], f32, kind="ExternalOutput")

    with tile.TileContext(nc) as tc, ExitStack() as ctx:
        consts = ctx.enter_context(tc.tile_pool(name="consts", bufs=1))
        jin = ctx.enter_context(tc.tile_pool(name="jin", bufs=2))
        x12 = ctx.enter_context(tc.tile_pool(name="x12", bufs=2))
        bigx = ctx.enter_context(tc.tile_pool(name="bigx", bufs=2))
        bigy = ctx.enter_context(tc.tile_pool(name="bigy", bufs=2))
        scr = ctx.enter_context(tc.tile_pool(name="scr", bufs=2))
        mxp = ctx.enter_context(tc.tile_pool(name="mxp", bufs=2))
        aggs = ctx.enter_context(tc.tile_pool(name="aggs", bufs=2))
        mm = ctx.enter_context(tc.tile_pool(name="mm", bufs=2, space="PSUM"))
        acc = ctx.enter_context(tc.tile_pool(name="acc", bufs=1, space="PSUM"))

        def const_tile(name, dram, shape, dt):
            t = consts.tile(shape, dt, tag=name)
            nc.sync.dma_start(t[:], dram.ap())
            return t

        w1t = const_tile("w1", w1_d, [FJ, H], bf16)
        w2t = const_tile("w2", w2_d, [H, H], bf16)
        w3t = const_tile("w3", w3_d, [H, H], bf16)
        wzt = const_tile("wz", wz_d, [H, H], bf16)
        w4t = const_tile("w4", w4_d, [H, H], bf16)
        w5t = const_tile("w5", w5_d, [H, H], bf16)
        ip_t = const_tile("ip", identp_d, [H, H], bf16)
        bv = const_tile("bv", bv_d, [H, 8], f32)

        def r3(ap, k):
            return ap.rearrange("p (k c) -> p k c", k=k)

        # square via DVE tensor_scalar pow: hits the 4x bf16 perf mode
        # (tensor_tensor mult only reaches ~1 elem/cycle). Inputs are
        # relu outputs (>= 0) so pow is safe.
        def square(dst, src):
            if USE_POW_SQUARE:
                nc.vector.tensor_scalar(dst, src, 2.0, None, ALU.pow)
            else:
                nc.vector.tensor_tensor(dst, src, src, ALU.mult)

        # ---- jets side: 4 layers as a list of emission steps (one
        # 1024-col layer-chunk each). Steps of group g+1 are interleaved
        # into group g's pair-chunk loop: PE and Scalar advance the next
        # group's jets while Vector drains this group's pair work, so
        # Vector rolls straight into the next y1 with z already ready.
        def make_jets(g, cap, jets_off):
            JCg = g * cap
            jt = jin.tile([FJ, JCg], bf16, tag="jt")
            x1 = x12.tile([H, JCg], bf16, tag="x1")
            x2 = x12.tile([H, JCg], bf16, tag="x2")
            x = bigx.tile([H, JCg], bf16, tag="x")
            z = bigx.tile([H, JCg], bf16, tag="z")

            def dma_step(jt=jt, jets_off=jets_off, JCg=JCg):
                nc.sync.dma_start(jt[:],
                                  jets_d.ap()[:, jets_off : jets_off + JCg])

            steps = [dma_step]
            plan = [(x1, w1t, jt, 0), (x2, w2t, x1, 1),
                    (x, w3t, x2, 2), (z, wzt, x, None)]
            for dst, wt, src, bias_col in plan:
                for c0 in range(0, JCg, 1024):
                    w = min(1024, JCg - c0)

                    def step(dst=dst, wt=wt, src=src, bias_col=bias_col,
                             c0=c0, w=w):
                        ps = mm.tile([H, 1024], f32, tag="mm")
                        for s0 in range(0, w, 512):
                            sw = min(512, w - s0)
                            nc.tensor.matmul(ps[:, s0 : s0 + sw], wt[:],
                                             src[:, c0 + s0 : c0 + s0 + sw],
                                             start=True, stop=True)
                        if bias_col is None:
                            nc.scalar.copy(dst[:, c0 : c0 + w], ps[:, :w])
                        else:
                            nc.scalar.activation(
                                dst[:, c0 : c0 + w], ps[:, :w], AF.Relu,
                                bias=bv[:, bias_col : bias_col + 1])

                    steps.append(step)
            return steps, x, z

        ev_off = 0
        pending_tail = [None]
        g0, cap0 = groups[0]
        cur_steps, cur_x, cur_z = make_jets(g0, cap0, 0)
        for st in cur_steps:
            st()
        jets_off = g0 * cap0
        for gi, (g, cap) in enumerate(groups):
            assert cap % 8 == 0 and cap <= 512
            JCg = g * cap
            prs = pairs_of(g)
            PG = len(prs)
            x, z = cur_x, cur_z

            if gi + 1 < len(groups):
                g2, cap2 = groups[gi + 1]
                nxt_steps, nxt_x, nxt_z = make_jets(g2, cap2, jets_off)
                jets_off += g2 * cap2
            else:
                nxt_steps, nxt_x, nxt_z = [], None, None
            squeue = list(nxt_steps)

            agg6 = aggs.tile([H, 6 * cap], f32, tag="agg6")

            def max_tree(src_tile, nslices, dst_f32, tag, l1_pool=False,
                         cap=cap):
                m, cur, off = nslices, src_tile, 0
                if m == 1:
                    nc.vector.tensor_copy(dst_f32, src_tile[:, 0:cap])
                    return
                first = True
                while m > 1:
                    k2 = (m + 1) // 2
                    if k2 == 1:
                        nxt = dst_f32
                    else:
                        nxt = mxp.tile([H, k2 * cap], bf16, tag=tag)
                    a0 = cur[:, off : off + k2 * cap]
                    a1 = cur[:, off + (m - k2) * cap : off + m * cap]
                    nc.vector.tensor_tensor(r3(nxt[:, 0 : k2 * cap], k2),
                                            r3(a0, k2), r3(a1, k2), ALU.max)
                    cur, off, m = nxt, 0, k2
                    first = False

            # ---- pairs side: SC-slice chunks, lag-1 ysq+sums.
            SC = max(1, 2048 // cap)
            PCH = SC * cap
            y3 = bigy.tile([H, PG * cap], bf16, tag="y3")
            # one PSUM bank per accumulator: sharing a bank between two
            # accumulation chains corrupts results (start=True resets the
            # whole bank, measured rel-err 0.22).
            a_sy = acc.tile([H, cap], f32, tag="a_sy")
            a_qy = acc.tile([H, cap], f32, tag="a_qy")
            a_sx = acc.tile([H, cap], f32, tag="a_sx")
            a_qx = acc.tile([H, cap], f32, tag="a_qx")

            def ysq_sums(p0, k, cap=cap, PCH=PCH, y3=y3, PG=PG,
                         a_sy=a_sy, a_qy=a_qy):
                w = k * cap
                ysq = scr.tile([H, PCH], bf16, tag="ysq")
                square(ysq[:, :w], y3[:, p0 * cap : p0 * cap + w])
                for s in range(k):
                    sl = p0 + s
                    nc.tensor.matmul(a_sy[:, :cap], ip_t[:],
                                     y3[:, sl * cap : (sl + 1) * cap],
                                     start=(sl == 0), stop=(sl == PG - 1))
                for s in range(k):
                    sl = p0 + s
                    nc.tensor.matmul(a_qy[:, :cap], ip_t[:],
                                     ysq[:, s * cap : (s + 1) * cap],
                                     start=(sl == 0), stop=(sl == PG - 1))

            prev = None
            y3_i = [0]
            chunk_starts = list(range(0, PG, SC))
            for ci, p0 in enumerate(chunk_starts):
                k = min(SC, PG - p0)
                w = k * cap
                y1 = scr.tile([H, PCH], bf16, tag="y1")
                s = 0
                while s < k:
                    i = prs[p0 + s][0]
                    r = 1
                    while s + r < k and prs[p0 + s + r][0] == i:
                        r += 1
                    j0 = prs[p0 + s][1]
                    nc.vector.tensor_tensor(
                        r3(y1[:, s * cap : (s + r) * cap], r),
                        r3(z[:, i * cap : (i + 1) * cap], 1).broadcast_to(
                            [H, r, cap]),
                        r3(z[:, j0 * cap : (j0 + r) * cap], r),
                        ALU.add)
                    s += r
                nc.vector.tensor_scalar(y1[:, :w], y1[:, :w], bv[:, 3:4],
                                        0.0, ALU.add, ALU.max)
                y2 = scr.tile([H, PCH], bf16, tag="y2")
                for n0 in range(0, w, 1024):
                    cw = min(1024, w - n0)
                    ps = mm.tile([H, 1024], f32, tag="mm")
                    for s0 in range(0, cw, 512):
                        sw = min(512, cw - s0)
                        nc.tensor.matmul(ps[:, s0 : s0 + sw], w4t[:],
                                         y1[:, n0 + s0 : n0 + s0 + sw],
                                         start=True, stop=True)
                    nc.scalar.activation(y2[:, n0 : n0 + cw], ps[:, :cw],
                                         AF.Relu, bias=bv[:, 4:5])
                for n0 in range(0, w, 1024):
                    cw = min(1024, w - n0)
                    ps = mm.tile([H, 1024], f32, tag="mm")
                    for s0 in range(0, cw, 512):
                        sw = min(512, cw - s0)
                        nc.tensor.matmul(ps[:, s0 : s0 + sw], w5t[:],
                                         y2[:, n0 + s0 : n0 + s0 + sw],
                                         start=True, stop=True)
                    dst = y3[:, p0 * cap + n0 : p0 * cap + n0 + cw]
                    if y3_i[0] % 2 == 0:
                        nc.scalar.activation(dst, ps[:, :cw], AF.Relu,
                                             bias=bv[:, 5:6])
                    else:
                        nc.vector.tensor_scalar(dst, ps[:, :cw], bv[:, 5:6],
                                                0.0, ALU.add, ALU.max)
                    y3_i[0] += 1
                if prev is not None:
                    ysq_sums(*prev)
                prev = (p0, k)
                # Pace the next group's jets steps across this group's
                # remaining chunks (all leftovers drain after the loop).
                rem = len(chunk_starts) - 1 - ci
                if rem > 0:
                    npop = -(-len(squeue) // (rem + 1))
                    for _ in range(min(npop, len(squeue))):
                        squeue.pop(0)()
                # The previous group's tail goes one chunk deep into this
                # group: V starts this group's y1 immediately (unblocking
                # PE and Scalar), and the tail fills V/PE slack while the
                # first chunk's matmuls run.
                if pending_tail[0] is not None:
                    pending_tail[0]()
                    pending_tail[0] = None

            for st in squeue:
                st()

            # x-side square/max emitted after the pair chunks: the Vector
            # engine runs them while PE finishes the pair matmuls, instead
            # of delaying the first y1 (which PE waits on).
            xsq = bigx.tile([H, JCg], bf16, tag="xsq")
            square(xsq[:], x[:])
            max_tree(x, g, agg6[:, 2 * cap : 3 * cap], "xt")

            def tail(prev=prev, y3=y3, PG=PG, g=g, cap=cap, agg6=agg6,
                     a_sy=a_sy, a_qy=a_qy, a_sx=a_sx, a_qx=a_qx, xsq=xsq,
                     x=x, ev_off=ev_off, max_tree=max_tree,
                     ysq_sums=ysq_sums):
                ysq_sums(*prev)
                for s in range(g):
                    nc.tensor.matmul(a_sx[:, :cap], ip_t[:],
                                     x[:, s * cap : (s + 1) * cap],
                                     start=(s == 0), stop=(s == g - 1))
                for s in range(g):
                    nc.tensor.matmul(a_qx[:, :cap], ip_t[:],
                                     xsq[:, s * cap : (s + 1) * cap],
                                     start=(s == 0), stop=(s == g - 1))
                max_tree(y3, PG, agg6[:, 5 * cap : 6 * cap], "yt",
                         l1_pool=True)
                nc.scalar.copy(agg6[:, 0:cap], a_sx[:, :cap])
                nc.scalar.copy(agg6[:, cap : 2 * cap], a_qx[:, :cap])
                nc.scalar.copy(agg6[:, 3 * cap : 4 * cap], a_sy[:, :cap])
                nc.scalar.copy(agg6[:, 4 * cap : 5 * cap], a_qy[:, :cap])
                nc.sync.dma_start(
                    out6_d.ap()[:, 6 * ev_off : 6 * ev_off + 6 * cap],
                    agg6[:])

            pending_tail[0] = tail

            cur_steps, cur_x, cur_z = nxt_steps, nxt_x, nxt_z
            ev_off += cap
        pending_tail[0]()

    nc.compile()
    return nc


# ---------------- host-side math ----------------

BN_EPS = 1e-3


def fold_params(inp):
    """Fold normalization + BN into per-layer (W, b). All numpy fp32."""
    mean_j = np.asarray(inp["mean_jets"], np.float32)
    std_j = np.asarray(inp["std_jets"], np.float32)
    w1f = np.asarray(inp["w1_first"], np.float32)
    w1r = np.asarray(inp["w1_rest"], np.float32)
    bn1 = np.asarray(inp["bn1"], np.float32)  # [3,4,H]: gamma, beta, mean, var
    w2f = np.asarray(inp["w2_first"], np.float32)
    w2r = np.asarray(inp["w2_rest"], np.float32)
    bn2 = np.asarray(inp["bn2"], np.float32)

    def bn_sb(row):
        gm, bt, mu, vv = row[0], row[1], row[2], row[3]
        s = gm / np.sqrt(vv + BN_EPS)
        return s.astype(np.float32), (bt - mu * s).astype(np.float32)

    s11, t11 = bn_sb(bn1[0]); s12, t12 = bn_sb(bn1[1]); s13, t13 = bn_sb(bn1[2])
    s21, t21 = bn_sb(bn2[0]); s22, t22 = bn_sb(bn2[1]); s23, t23 = bn_sb(bn2[2])

    A = w1f / std_j[:, None]
    c = -(mean_j / std_j) @ w1f
    return dict(
        W1=A * s11[None, :], b1=c * s11 + t11,
        W2=w1r[0] * s12[None, :], b2=t12,
        W3=w1r[1] * s13[None, :], b3=t13,
        Wz=w2f * s21[None, :], bz=t21,
        W4=w2r[0] * s22[None, :], b4=t22,
        W5=w2r[1] * s23[None, :], b5=t23,
    )


# ---------------- full kernel entry point ----------------

N_CORES = 8

_cache = {}
_TRACE = [False]
_LAST_RESULT = [None]


def _get_program(groups_key):
    if groups_key not in _cache:
        _cache[groups_key] = build_program(list(groups_key))
    return _cache[groups_key]


def _np_dt(dt):
    return mybir.dt.np(dt)


def _plan(n):
    """Returns (groups, slots): groups = [(g, cap)], slots[c][gi] =
    (padded index array, real count) for core c, group gi."""
    gs = []
    idx_by_g = {}
    for g in range(2, 11):
        idx = np.nonzero(n == g)[0]
        if len(idx):
            gs.append(g)
            idx_by_g[g] = idx
    stray = np.nonzero((n < 2) | (n > 10))[0]
    if len(stray):
        if not gs:
            gs.append(2)
            idx_by_g[2] = stray
        else:
            idx_by_g[gs[-1]] = np.concatenate([idx_by_g[gs[-1]], stray])
    # Descending size order: each group's jets chain is emitted inside
    # the previous (bigger) group's pair phase, so it pipelines fully.
    gs = sorted(gs, key=lambda g: -g)
    groups = []
    slots = [[] for _ in range(N_CORES)]
    for g in gs:
        idx = idx_by_g[g]
        per_core = [idx[c::N_CORES] for c in range(N_CORES)]
        mx = max(len(p) for p in per_core)
        cap = max(8, ((mx + 7) // 8) * 8)
        groups.append((g, cap))
        fill = idx[0]
        for c in range(N_CORES):
            p = per_core[c]
            pad = np.full(cap, p[0] if len(p) else fill, dtype=np.int64)
            pad[: len(p)] = p
            slots[c].append((pad, len(p)))
    return groups, slots


def _pack_jets(jets, groups, slots_c):
    cols = []
    for (g, cap), (ids, _cnt) in zip(groups, slots_c):
        ev = jets[ids][:, :g, :]  # [cap, g, 16]
        cols.append(np.ascontiguousarray(ev.transpose(2, 1, 0)).reshape(
            FJ, g * cap))
    return np.concatenate(cols, axis=1).astype(_np_dt(bf16), copy=False)


def kernel(**inputs):
    from concourse.bass_utils import run_bass_kernel_spmd

    jets = np.asarray(inputs["inputs_jets"], dtype=np.float32)
    B = jets.shape[0]
    mask = (jets != 0.0).any(-1)
    n = mask.sum(-1).astype(np.int64)
    # compact valid jets to the front (no-op for the standard generator)
    if not np.array_equal(mask, np.arange(jets.shape[1])[None, :] < n[:, None]):
        order = np.argsort(~mask, axis=1, kind="stable")
        jets = np.take_along_axis(jets, order[:, :, None], axis=1)

    P = fold_params(inputs)
    groups, slots = _plan(n)
    nc = _get_program(tuple(groups))

    bvec = np.zeros((H, 8), np.float32)
    for i, k in enumerate(["b1", "b2", "b3", "bz", "b4", "b5"]):
        bvec[:, i] = P[k]
    ident = np.eye(H, dtype=np.float32)
    bnp = _np_dt(bf16)
    common = {
        "w1": P["W1"].astype(bnp), "w2": P["W2"].astype(bnp),
        "w3": P["W3"].astype(bnp), "wz": P["Wz"].astype(bnp),
        "w4": P["W4"].astype(bnp), "w5": P["W5"].astype(bnp),
        "identp": ident.astype(bnp), "bvec": bvec,
    }
    in_maps = []
    for c in range(N_CORES):
        m = dict(common)
        m["jets"] = _pack_jets(jets, groups, slots[c])
        in_maps.append(m)

    res = run_bass_kernel_spmd(nc, in_maps, core_ids=list(range(N_CORES)),
                               trace=_TRACE[0])
    _LAST_RESULT[0] = res

    agg_x = np.empty((B, 4 * H), np.float32)
    agg_y = np.empty((B, 4 * H), np.float32)
    for c in range(N_CORES):
        o6 = res.results[c]["out6"]  # [H, 6*EC] f32
        ev_off = 0
        for (g, cap), (ids, cnt) in zip(groups, slots[c]):
            blk = o6[:, 6 * ev_off : 6 * ev_off + 6 * cap]
            sx = blk[:, 0:cap].T[:cnt]
            qx = blk[:, cap : 2 * cap].T[:cnt]
            mx = blk[:, 2 * cap : 3 * cap].T[:cnt]
            sy = blk[:, 3 * cap : 4 * cap].T[:cnt]
            qy = blk[:, 4 * cap : 5 * cap].T[:cnt]
            my = blk[:, 5 * cap : 6 * cap].T[:cnt]
            ii = ids[:cnt]
            mean_x = sx / g
            mean_y = sy / (g * (g - 1) // 2)
            agg_x[ii] = np.concatenate(
                [sx, mx, mean_x, qx / g - mean_x * mean_x], axis=1)
            agg_y[ii] = np.concatenate(
                [sy, my, mean_y, qy / (g * (g - 1) // 2) - mean_y * mean_y],
                axis=1)
            ev_off += cap
    return agg_x, agg_y


# revision 57
# speedup vs baseline: 1.0934x; 1.0915x over previous
# DeepSet Trainium2 kernel.
#
# Strategy: events are sorted by jet-count n (2..10) on the host and
# round-robin sharded across 8 cores into per-group slots of capacity cap_g
# (multiple of 8, exact-packed). Within a group every event has exactly n=g
# valid jets, so all masks, pair structures and aggregation counts are
# compile-time constants.
#
# Math folding (host, O(params)):
#   every Dense+BN+relu block becomes relu(h @ W' + b') with W', b' folded.
#   MLP2 layer 1 uses the z-trick: y1 = relu(z_i + z_j + t) with z = x @ Wz'.
#   t is folded into the y1 relu pass (tensor_scalar add-bias+max0).
#
# Device layout: feature-major [H=128 partitions, columns = slice*cap + b]
# per group, all activations bf16 (PE: 1 col/cycle), PSUM f32.
# Work distribution across engines:
#   PE:   all matmuls + Sum/Sumsq of pairs via PSUM-accumulating identity mms
#   Act:  x1/x2/z/x PSUM evacs (relu+bias / copy), y2 evac, xsq (Square)
#   DVE:  x-side sum/sumsq/max trees, y1 add (broadcast AP) + relu,
#         ysq (y3*y3), y-side max tree
#   Pool: y3 PSUM evac (tensor_scalar bias+relu)
# Mean/Var and the final [events, 4H] transpose are computed on the HOST
# from the 6 DMA'd feature-major aggregates (sum/sumsq/max per side).
import math
from contextlib import ExitStack

import numpy as np

import concourse.bass as bass
import concourse.bacc as bacc
import concourse.tile as tile
import concourse.mybir as mybir

f32 = mybir.dt.float32
bf16 = mybir.dt.bfloat16
AF = mybir.ActivationFunctionType
ALU = mybir.AluOpType

H = 128
FJ = 16


def pairs_of(g):
    return [(i, j) for i in range(g) for j in range(i + 1, g)]


# GPSIMD findings (measured): cannot access PSUM (birverifier), has no
# max opcode, tensor_scalar runs ~12ns/col, and even its decent-rate
# tensor_tensor mult slows the Vector engine ~20-30% via SBUF port
# contention. Net negative everywhere -> unused.
# pow is rejected by the DVE ISA check (tensor_scalar_valid_ops).
USE_POW_SQUARE = False


def build_program(groups):
    """groups: list of (g, cap) with cap a multiple of 8, cap <= 512."""
    JC = sum(g * cap for g, cap in groups)
    EC = sum(cap for _, cap in groups)

    nc = bacc.Bacc("TRN2", target_bir_lowering=False, debug=False)

    jets_d = nc.dram_tensor("jets", [FJ, JC], bf16, kind="ExternalInput")
    w1_d = nc.dram_tensor("w1", [FJ, H], bf16, kind="ExternalInput")
    w2_d = nc.dram_tensor("w2", [H, H], bf16, kind="ExternalInput")
    w3_d = nc.dram_tensor("w3", [H, H], bf16, kind="ExternalInput")
    wz_d = nc.dram_tensor("wz", [H, H], bf16, kind="ExternalInput")
    w4_d = nc.dram_tensor("w4", [H, H], bf16, kind="ExternalInput")
    w5_d = nc.dram_tensor("w5", [H, H], bf16, kind="ExternalInput")
    identp_d = nc.dram_tensor("identp", [H, H], bf16, kind="ExternalInput")
    # bias vector cols: 0..5 = b1, b2, b3, t(=bz), b4, b5
    bv_d = nc.dram_tensor("bvec", [H, 8], f32, kind="ExternalInput")
    # per group: 6 aggregates [H, cap] each, packed [sx qx mx sy qy my]
    out6_d = nc.dram_tensor("out6", [H, 6 * EC], f32, kind="ExternalOutput")

    with tile.TileContext(nc) as tc, ExitStack() as ctx:
        consts = ctx.enter_context(tc.tile_pool(name="consts", bufs=1))
        jin = ctx.enter_context(tc.tile_pool(name="jin", bufs=2))
        x12 = ctx.enter_context(tc.tile_pool(name="x12", bufs=2))
        bigx = ctx.enter_context(tc.tile_pool(name="bigx", bufs=2))
        bigy = ctx.enter_context(tc.tile_pool(name="bigy", bufs=2))
        scr = ctx.enter_context(tc.tile_pool(name="scr", bufs=2))
        mxp = ctx.enter_context(tc.tile_pool(name="mxp", bufs=2))
        aggs = ctx.enter_context(tc.tile_pool(name="aggs", bufs=2))
        mm = ctx.enter_context(tc.tile_pool(name="mm", bufs=3, space="PSUM"))
        acc = ctx.enter_context(tc.tile_pool(name="acc", bufs=1, space="PSUM"))

        def const_tile(name, dram, shape, dt):
            t = consts.tile(shape, dt, tag=name)
            nc.sync.dma_start(t[:], dram.ap())
            return t

        w1t = const_tile("w1", w1_d, [FJ, H], bf16)
        w2t = const_tile("w2", w2_d, [H, H], bf16)
        w3t = const_tile("w3", w3_d, [H, H], bf16)
        wzt = const_tile("wz", wz_d, [H, H], bf16)
        w4t = const_tile("w4", w4_d, [H, H], bf16)
        w5t = const_tile("w5", w5_d, [H, H], bf16)
        ip_t = const_tile("ip", identp_d, [H, H], bf16)
        bv = const_tile("bv", bv_d, [H, 8], f32)

        def r3(ap, k):
            return ap.rearrange("p (k c) -> p k c", k=k)

        # square via DVE tensor_scalar pow: hits the 4x bf16 perf mode
        # (tensor_tensor mult only reaches ~1 elem/cycle). Inputs are
        # relu outputs (>= 0) so pow is safe.
        def square(dst, src):
            if USE_POW_SQUARE:
                nc.vector.tensor_scalar(dst, src, 2.0, None, ALU.pow)
            else:
                nc.vector.tensor_tensor(dst, src, src, ALU.mult)

        # ---- jets side: 4 layers as a list of emission steps (one
        # 1024-col layer-chunk each). Steps of group g+1 are interleaved
        # into group g's pair-chunk loop: PE and Scalar advance the next
        # group's jets while Vector drains this group's pair work, so
        # Vector rolls straight into the next y1 with z already ready.
        def make_jets(g, cap, jets_off):
            JCg = g * cap
            jt = jin.tile([FJ, JCg], bf16, tag="jt")
            x1 = x12.tile([H, JCg], bf16, tag="x1")
            x2 = x12.tile([H, JCg], bf16, tag="x2")
            x = bigx.tile([H, JCg], bf16, tag="x")
            z = bigx.tile([H, JCg], bf16, tag="z")

            def dma_step(jt=jt, jets_off=jets_off, JCg=JCg):
                nc.sync.dma_start(jt[:],
                                  jets_d.ap()[:, jets_off : jets_off + JCg])

            steps = [dma_step]
            plan = [(x1, w1t, jt, 0), (x2, w2t, x1, 1),
                    (x, w3t, x2, 2), (z, wzt, x, None)]
            for dst, wt, src, bias_col in plan:
                for c0 in range(0, JCg, 1024):
                    w = min(1024, JCg - c0)

                    def step(dst=dst, wt=wt, src=src, bias_col=bias_col,
                             c0=c0, w=w):
                        ps = mm.tile([H, 1024], f32, tag="mm")
                        for s0 in range(0, w, 512):
                            sw = min(512, w - s0)
                            nc.tensor.matmul(ps[:, s0 : s0 + sw], wt[:],
                                             src[:, c0 + s0 : c0 + s0 + sw],
                                             start=True, stop=True)
                        if bias_col is None:
                            nc.scalar.copy(dst[:, c0 : c0 + w], ps[:, :w])
                        else:
                            nc.scalar.activation(
                                dst[:, c0 : c0 + w], ps[:, :w], AF.Relu,
                                bias=bv[:, bias_col : bias_col + 1])

                    steps.append(step)
            return steps, x, z

        ev_off = 0
        pending_tail = [None]
        g0, cap0 = groups[0]
        cur_steps, cur_x, cur_z = make_jets(g0, cap0, 0)
        for st in cur_steps:
            st()
        jets_off = g0 * cap0
        for gi, (g, cap) in enumerate(groups):
            assert cap % 8 == 0 and cap <= 512
            JCg = g * cap
            prs = pairs_of(g)
            PG = len(prs)
            x, z = cur_x, cur_z

            if gi + 1 < len(groups):
                g2, cap2 = groups[gi + 1]
                nxt_steps, nxt_x, nxt_z = make_jets(g2, cap2, jets_off)
                jets_off += g2 * cap2
            else:
                nxt_steps, nxt_x, nxt_z = [], None, None
            squeue = list(nxt_steps)

            agg6 = aggs.tile([H, 6 * cap], f32, tag="agg6")

            def sum_tree(src_tile, nslices, dst_f32, cap):
                m, cur, off = nslices, src_tile, 0
                if m == 1:
                    nc.vector.tensor_copy(dst_f32, cur[:, 0:cap])
                    return
                while m > 1:
                    k2 = m // 2
                    if k2 == 1:
                        nxt = dst_f32
                    else:
                        nxt = mxp.tile([H, k2 * cap], bf16, tag="xt")
                    nc.vector.tensor_tensor(
                        r3(nxt[:, 0 : k2 * cap], k2),
                        r3(cur[:, off : off + k2 * cap], k2),
                        r3(cur[:, off + k2 * cap : off + 2 * k2 * cap], k2),
                        ALU.add)
                    if m % 2:
                        nc.vector.tensor_add(
                            nxt[:, 0:cap], nxt[:, 0:cap],
                            cur[:, off + (m - 1) * cap : off + m * cap])
                    cur, off, m = nxt, 0, k2

            def max_tree(src_tile, nslices, dst_f32, tag, l1_pool=False,
                         cap=cap):
                m, cur, off = nslices, src_tile, 0
                if m == 1:
                    nc.vector.tensor_copy(dst_f32, src_tile[:, 0:cap])
                    return
                first = True
                while m > 1:
                    k2 = (m + 1) // 2
                    if k2 == 1:
                        nxt = dst_f32
                    else:
                        nxt = mxp.tile([H, k2 * cap], bf16, tag=tag)
                    a0 = cur[:, off : off + k2 * cap]
                    a1 = cur[:, off + (m - k2) * cap : off + m * cap]
                    nc.vector.tensor_tensor(r3(nxt[:, 0 : k2 * cap], k2),
                                            r3(a0, k2), r3(a1, k2), ALU.max)
                    cur, off, m = nxt, 0, k2
                    first = False

            # ---- pairs side: SC-slice chunks, lag-1 ysq+sums.
            SC = max(1, 2048 // cap)
            PCH = SC * cap
            y3 = bigy.tile([H, PG * cap], bf16, tag="y3")
            # one PSUM bank per accumulator: sharing a bank between two
            # accumulation chains corrupts results (start=True resets the
            # whole bank, measured rel-err 0.22). x-side sums use DVE
            # trees instead, freeing two banks for a deeper mm pool.
            a_sy = acc.tile([H, cap], f32, tag="a_sy")
            a_qy = acc.tile([H, cap], f32, tag="a_qy")

            def ysq_sums(p0, k, cap=cap, PCH=PCH, y3=y3, PG=PG,
                         a_sy=a_sy, a_qy=a_qy):
                w = k * cap
                ysq = scr.tile([H, PCH], bf16, tag="ysq")
                square(ysq[:, :w], y3[:, p0 * cap : p0 * cap + w])
                for s in range(k):
                    sl = p0 + s
                    nc.tensor.matmul(a_sy[:, :cap], ip_t[:],
                                     y3[:, sl * cap : (sl + 1) * cap],
                                     start=(sl == 0), stop=(sl == PG - 1))
                for s in range(k):
                    sl = p0 + s
                    nc.tensor.matmul(a_qy[:, :cap], ip_t[:],
                                     ysq[:, s * cap : (s + 1) * cap],
                                     start=(sl == 0), stop=(sl == PG - 1))

            prev = None
            y3_i = [0]
            chunk_starts = list(range(0, PG, SC))
            for ci, p0 in enumerate(chunk_starts):
                k = min(SC, PG - p0)
                w = k * cap
                y1 = scr.tile([H, PCH], bf16, tag="y1")
                s = 0
                while s < k:
                    i = prs[p0 + s][0]
                    r = 1
                    while s + r < k and prs[p0 + s + r][0] == i:
                        r += 1
                    j0 = prs[p0 + s][1]
                    nc.vector.tensor_tensor(
                        r3(y1[:, s * cap : (s + r) * cap], r),
                        r3(z[:, i * cap : (i + 1) * cap], 1).broadcast_to(
                            [H, r, cap]),
                        r3(z[:, j0 * cap : (j0 + r) * cap], r),
                        ALU.add)
                    s += r
                nc.vector.tensor_scalar(y1[:, :w], y1[:, :w], bv[:, 3:4],
                                        0.0, ALU.add, ALU.max)
                y2 = scr.tile([H, PCH], bf16, tag="y2")
                for n0 in range(0, w, 1024):
                    cw = min(1024, w - n0)
                    ps = mm.tile([H, 1024], f32, tag="mm")
                    for s0 in range(0, cw, 512):
                        sw = min(512, cw - s0)
                        nc.tensor.matmul(ps[:, s0 : s0 + sw], w4t[:],
                                         y1[:, n0 + s0 : n0 + s0 + sw],
                                         start=True, stop=True)
                    nc.scalar.activation(y2[:, n0 : n0 + cw], ps[:, :cw],
                                         AF.Relu, bias=bv[:, 4:5])
                for n0 in range(0, w, 1024):
                    cw = min(1024, w - n0)
                    ps = mm.tile([H, 1024], f32, tag="mm")
                    for s0 in range(0, cw, 512):
                        sw = min(512, cw - s0)
                        nc.tensor.matmul(ps[:, s0 : s0 + sw], w5t[:],
                                         y2[:, n0 + s0 : n0 + s0 + sw],
                                         start=True, stop=True)
                    dst = y3[:, p0 * cap + n0 : p0 * cap + n0 + cw]
                    if y3_i[0] % 2 == 0:
                        nc.scalar.activation(dst, ps[:, :cw], AF.Relu,
                                             bias=bv[:, 5:6])
                    else:
                        nc.vector.tensor_scalar(dst, ps[:, :cw], bv[:, 5:6],
                                                0.0, ALU.add, ALU.max)
                    y3_i[0] += 1
                if prev is not None:
                    ysq_sums(*prev)
                prev = (p0, k)
                # Pace the next group's jets steps across this group's
                # remaining chunks (all leftovers drain after the loop).
                rem = len(chunk_starts) - 1 - ci
                if rem > 0:
                    npop = -(-len(squeue) // (rem + 1))
                    for _ in range(min(npop, len(squeue))):
                        squeue.pop(0)()
                # The previous group's tail goes one chunk deep into this
                # group: V starts this group's y1 immediately (unblocking
                # PE and Scalar), and the tail fills V/PE slack while the
                # first chunk's matmuls run.
                if pending_tail[0] is not None:
                    pending_tail[0]()
                    pending_tail[0] = None

            for st in squeue:
                st()

            # x-side square/trees emitted after the pair chunks: the Vector
            # engine runs them while PE finishes the pair matmuls, instead
            # of delaying the first y1 (which PE waits on).
            xsq = bigx.tile([H, JCg], bf16, tag="xsq")
            square(xsq[:], x[:])
            sum_tree(x, g, agg6[:, 0:cap], cap)
            sum_tree(xsq, g, agg6[:, cap : 2 * cap], cap)
            max_tree(x, g, agg6[:, 2 * cap : 3 * cap], "xt")

            def tail(prev=prev, y3=y3, PG=PG, g=g, cap=cap, agg6=agg6,
                     a_sy=a_sy, a_qy=a_qy, ev_off=ev_off, max_tree=max_tree,
                     ysq_sums=ysq_sums):
                ysq_sums(*prev)
                max_tree(y3, PG, agg6[:, 5 * cap : 6 * cap], "yt",
                         l1_pool=True)
                nc.scalar.copy(agg6[:, 3 * cap : 4 * cap], a_sy[:, :cap])
                nc.scalar.copy(agg6[:, 4 * cap : 5 * cap], a_qy[:, :cap])
                nc.sync.dma_start(
                    out6_d.ap()[:, 6 * ev_off : 6 * ev_off + 6 * cap],
                    agg6[:])

            pending_tail[0] = tail

            cur_steps, cur_x, cur_z = nxt_steps, nxt_x, nxt_z
            ev_off += cap
        pending_tail[0]()

    nc.compile()
    return nc


# ---------------- host-side math ----------------

BN_EPS = 1e-3


def fold_params(inp):
    """Fold normalization + BN into per-layer (W, b). All numpy fp32."""
    mean_j = np.asarray(inp["mean_jets"], np.float32)
    std_j = np.asarray(inp["std_jets"], np.float32)
    w1f = np.asarray(inp["w1_first"], np.float32)
    w1r = np.asarray(inp["w1_rest"], np.float32)
    bn1 = np.asarray(inp["bn1"], np.float32)  # [3,4,H]: gamma, beta, mean, var
    w2f = np.asarray(inp["w2_first"], np.float32)
    w2r = np.asarray(inp["w2_rest"], np.float32)
    bn2 = np.asarray(inp["bn2"], np.float32)

    def bn_sb(row):
        gm, bt, mu, vv = row[0], row[1], row[2], row[3]
        s = gm / np.sqrt(vv + BN_EPS)
        return s.astype(np.float32), (bt - mu * s).astype(np.float32)

    s11, t11 = bn_sb(bn1[0]); s12, t12 = bn_sb(bn1[1]); s13, t13 = bn_sb(bn1[2])
    s21, t21 = bn_sb(bn2[0]); s22, t22 = bn_sb(bn2[1]); s23, t23 = bn_sb(bn2[2])

    A = w1f / std_j[:, None]
    c = -(mean_j / std_j) @ w1f
    return dict(
        W1=A * s11[None, :], b1=c * s11 + t11,
        W2=w1r[0] * s12[None, :], b2=t12,
        W3=w1r[1] * s13[None, :], b3=t13,
        Wz=w2f * s21[None, :], bz=t21,
        W4=w2r[0] * s22[None, :], b4=t22,
        W5=w2r[1] * s23[None, :], b5=t23,
    )


# ---------------- full kernel entry point ----------------

N_CORES = 8

_cache = {}
_TRACE = [False]
_LAST_RESULT = [None]


def _get_program(groups_key):
    if groups_key not in _cache:
        _cache[groups_key] = build_program(list(groups_key))
    return _cache[groups_key]


def _np_dt(dt):
    return mybir.dt.np(dt)


def _plan(n):
    """Returns (groups, slots): groups = [(g, cap)], slots[c][gi] =
    (padded index array, real count) for core c, group gi."""
    gs = []
    idx_by_g = {}
    for g in range(2, 11):
        idx = np.nonzero(n == g)[0]
        if len(idx):
            gs.append(g)
            idx_by_g[g] = idx
    stray = np.nonzero((n < 2) | (n > 10))[0]
    if len(stray):
        if not gs:
            gs.append(2)
            idx_by_g[2] = stray
        else:
            idx_by_g[gs[-1]] = np.concatenate([idx_by_g[gs[-1]], stray])
    # Descending size order: each group's jets chain is emitted inside
    # the previous (bigger) group's pair phase, so it pipelines fully.
    gs = sorted(gs, key=lambda g: -g)
    groups = []
    slots = [[] for _ in range(N_CORES)]
    for g in gs:
        idx = idx_by_g[g]
        per_core = [idx[c::N_CORES] for c in range(N_CORES)]
        mx = max(len(p) for p in per_core)
        cap = max(8, ((mx + 7) // 8) * 8)
        groups.append((g, cap))
        fill = idx[0]
        for c in range(N_CORES):
            p = per_core[c]
            pad = np.full(cap, p[0] if len(p) else fill, dtype=np.int64)
            pad[: len(p)] = p
            slots[c].append((pad, len(p)))
    return groups, slots


def _pack_jets(jets, groups, slots_c):
    cols = []
    for (g, cap), (ids, _cnt) in zip(groups, slots_c):
        ev = jets[ids][:, :g, :]  # [cap, g, 16]
        cols.append(np.ascontiguousarray(ev.transpose(2, 1, 0)).reshape(
            FJ, g * cap))
    return np.concatenate(cols, axis=1).astype(_np_dt(bf16), copy=False)


def kernel(**inputs):
    from concourse.bass_utils import run_bass_kernel_spmd

    jets = np.asarray(inputs["inputs_jets"], dtype=np.float32)
    B = jets.shape[0]
    mask = (jets != 0.0).any(-1)
    n = mask.sum(-1).astype(np.int64)
    # compact valid jets to the front (no-op for the standard generator)
    if not np.array_equal(mask, np.arange(jets.shape[1])[None, :] < n[:, None]):
        order = np.argsort(~mask, axis=1, kind="stable")
        jets = np.take_along_axis(jets, order[:, :, None], axis=1)

    P = fold_params(inputs)
    groups, slots = _plan(n)
    nc = _get_program(tuple(groups))

    bvec = np.zeros((H, 8), np.float32)
    for i, k in enumerate(["b1", "b2", "b3", "bz", "b4", "b5"]):
        bvec[:, i] = P[k]
    ident = np.eye(H, dtype=np.float32)
    bnp = _np_dt(bf16)
    common = {
        "w1": P["W1"].astype(bnp), "w2": P["W2"].astype(bnp),
        "w3": P["W3"].astype(bnp), "wz": P["Wz"].astype(bnp),
        "w4": P["W4"].astype(bnp), "w5": P["W5"].astype(bnp),
        "identp": ident.astype(bnp), "bvec": bvec,
    }
    in_maps = []
    for c in range(N_CORES):
        m = dict(common)
        m["jets"] = _pack_jets(jets, groups, slots[c])
        in_maps.append(m)

    res = run_bass_kernel_spmd(nc, in_maps, core_ids=list(range(N_CORES)),
                               trace=_TRACE[0])
    _LAST_RESULT[0] = res

    agg_x = np.empty((B, 4 * H), np.float32)
    agg_y = np.empty((B, 4 * H), np.float32)
    for c in range(N_CORES):
        o6 = res.results[c]["out6"]  # [H, 6*EC] f32
        ev_off = 0
        for (g, cap), (ids, cnt) in zip(groups, slots[c]):
            blk = o6[:, 6 * ev_off : 6 * ev_off + 6 * cap]
            sx = blk[:, 0:cap].T[:cnt]
            qx = blk[:, cap : 2 * cap].T[:cnt]
            mx = blk[:, 2 * cap : 3 * cap].T[:cnt]
            sy = blk[:, 3 * cap : 4 * cap].T[:cnt]
            qy = blk[:, 4 * cap : 5 * cap].T[:cnt]
            my = blk[:, 5 * cap : 6 * cap].T[:cnt]
            ii = ids[:cnt]
            mean_x = sx / g
            mean_y = sy / (g * (g - 1) // 2)
            agg_x[ii] = np.concatenate(
                [sx, mx, mean_x, qx / g - mean_x * mean_x], axis=1)
            agg_y[ii] = np.concatenate(
                [sy, my, mean_y, qy / (g * (g - 1) // 2) - mean_y * mean_y],
                axis=1)
            ev_off += cap
    return agg_x, agg_y


# revision 58
# speedup vs baseline: 1.1540x; 1.0555x over previous
# DeepSet Trainium2 kernel.
#
# Strategy: events are sorted by jet-count n (2..10) on the host and
# round-robin sharded across 8 cores into per-group slots of capacity cap_g
# (multiple of 8, exact-packed). Within a group every event has exactly n=g
# valid jets, so all masks, pair structures and aggregation counts are
# compile-time constants.
#
# Math folding (host, O(params)):
#   every Dense+BN+relu block becomes relu(h @ W' + b') with W', b' folded.
#   MLP2 layer 1 uses the z-trick: y1 = relu(z_i + z_j + t) with z = x @ Wz'.
#   t is folded into the y1 relu pass (tensor_scalar add-bias + max0).
#
# Device layout: feature-major [H=128 partitions, columns = slice*cap + b]
# per group, all activations bf16 (PE: 1 col/cycle), PSUM f32.
# The device computes ONLY the MLP chains:
#   jets:  x1 = relu(W1 jt + b1); x2 = relu(W2 x1 + b2); x = relu(W3 x2 + b3)
#          z = Wz x (plain copy evac)
#   pairs: y1 = relu(z_i + z_j + t) (DVE broadcast-add + tensor_scalar 4x)
#          y2 = relu(W4 y1 + b4); y3 = relu(W5 y2 + b5)
# and streams x and y3 (bf16) to DRAM. All aggregations (sum/max/mean/var
# over jets and pairs) happen on the HOST in f32 — the DMA engines were
# ~10% busy while Vector/Scalar were the kernel bottleneck, so shipping
# raw activations beats computing reductions on-device.
#
# Engine split (measured rates, ns/col of 128 rows):
#   Scalar  (~1.05): x1/x2/x/z PSUM evacs, y2 evac, y3 evac (2 of 3)
#   Vector  (~0.65 TT / 0.26 TSP-bf16 / 1.05 PSUM): y1 add, y1 relu,
#           y3 evac (1 of 3)
#   PE:     all matmuls (bf16 1 col/cycle, dual-buffered weight loads)
#   GPSIMD: unused — no PSUM access, no max opcode, slow tensor_scalar,
#           and its tensor_tensor traffic degrades Vector ~25% (measured).
#
# The next group's jets layer-chunks are emitted interleaved into the
# current group's pair-chunk loop (groups in descending size order), so
# PE/Scalar advance the next group while Vector drains the current one.
from contextlib import ExitStack

import numpy as np

import concourse.bass as bass
import concourse.bacc as bacc
import concourse.tile as tile
import concourse.mybir as mybir

f32 = mybir.dt.float32
bf16 = mybir.dt.bfloat16
AF = mybir.ActivationFunctionType
ALU = mybir.AluOpType

H = 128
FJ = 16


def pairs_of(g):
    return [(i, j) for i in range(g) for j in range(i + 1, g)]


def build_program(groups):
    """groups: list of (g, cap) with cap a multiple of 8, cap <= 512."""
    JC = sum(g * cap for g, cap in groups)
    YC = sum((g * (g - 1) // 2) * cap for g, cap in groups)

    nc = bacc.Bacc("TRN2", target_bir_lowering=False, debug=False)

    jets_d = nc.dram_tensor("jets", [FJ, JC], bf16, kind="ExternalInput")
    w1_d = nc.dram_tensor("w1", [FJ, H], bf16, kind="ExternalInput")
    w2_d = nc.dram_tensor("w2", [H, H], bf16, kind="ExternalInput")
    w3_d = nc.dram_tensor("w3", [H, H], bf16, kind="ExternalInput")
    wz_d = nc.dram_tensor("wz", [H, H], bf16, kind="ExternalInput")
    w4_d = nc.dram_tensor("w4", [H, H], bf16, kind="ExternalInput")
    w5_d = nc.dram_tensor("w5", [H, H], bf16, kind="ExternalInput")
    # bias vector cols: 0..5 = b1, b2, b3, t(=bz), b4, b5
    bv_d = nc.dram_tensor("bvec", [H, 8], f32, kind="ExternalInput")
    xout_d = nc.dram_tensor("xout", [H, JC], bf16, kind="ExternalOutput")
    yout_d = nc.dram_tensor("yout", [H, YC], bf16, kind="ExternalOutput")

    with tile.TileContext(nc) as tc, ExitStack() as ctx:
        consts = ctx.enter_context(tc.tile_pool(name="consts", bufs=1))
        jin = ctx.enter_context(tc.tile_pool(name="jin", bufs=2))
        x12 = ctx.enter_context(tc.tile_pool(name="x12", bufs=2))
        bigx = ctx.enter_context(tc.tile_pool(name="bigx", bufs=2))
        scr = ctx.enter_context(tc.tile_pool(name="scr", bufs=3))
        mm = ctx.enter_context(tc.tile_pool(name="mm", bufs=4, space="PSUM"))

        def const_tile(name, dram, shape, dt):
            t = consts.tile(shape, dt, tag=name)
            nc.sync.dma_start(t[:], dram.ap())
            return t

        w1t = const_tile("w1", w1_d, [FJ, H], bf16)
        w2t = const_tile("w2", w2_d, [H, H], bf16)
        w3t = const_tile("w3", w3_d, [H, H], bf16)
        wzt = const_tile("wz", wz_d, [H, H], bf16)
        w4t = const_tile("w4", w4_d, [H, H], bf16)
        w5t = const_tile("w5", w5_d, [H, H], bf16)
        bv = const_tile("bv", bv_d, [H, 8], f32)

        def r3(ap, k):
            return ap.rearrange("p (k c) -> p k c", k=k)

        # ---- jets side: 4 layers as a list of emission steps (one
        # 1024-col layer-chunk each). Steps of group g+1 are interleaved
        # into group g's pair-chunk loop.
        def make_jets(g, cap, jets_off):
            JCg = g * cap
            jt = jin.tile([FJ, JCg], bf16, tag="jt")
            x1 = x12.tile([H, JCg], bf16, tag="x1")
            x2 = x12.tile([H, JCg], bf16, tag="x2")
            x = bigx.tile([H, JCg], bf16, tag="x")
            z = bigx.tile([H, JCg], bf16, tag="z")

            def dma_in(jt=jt, jets_off=jets_off, JCg=JCg):
                nc.sync.dma_start(jt[:],
                                  jets_d.ap()[:, jets_off : jets_off + JCg])

            def dma_x(x=x, jets_off=jets_off, JCg=JCg):
                nc.sync.dma_start(
                    xout_d.ap()[:, jets_off : jets_off + JCg], x[:])

            steps = [dma_in]
            plan = [(x1, w1t, jt, 0), (x2, w2t, x1, 1),
                    (x, w3t, x2, 2), (z, wzt, x, None)]
            for li, (dst, wt, src, bias_col) in enumerate(plan):
                for c0 in range(0, JCg, 1024):
                    w = min(1024, JCg - c0)

                    def step(dst=dst, wt=wt, src=src, bias_col=bias_col,
                             c0=c0, w=w):
                        ps = mm.tile([H, 1024], f32, tag="mm")
                        for s0 in range(0, w, 512):
                            sw = min(512, w - s0)
                            nc.tensor.matmul(ps[:, s0 : s0 + sw], wt[:],
                                             src[:, c0 + s0 : c0 + s0 + sw],
                                             start=True, stop=True)
                        if bias_col is None:
                            nc.scalar.copy(dst[:, c0 : c0 + w], ps[:, :w])
                        else:
                            nc.scalar.activation(
                                dst[:, c0 : c0 + w], ps[:, :w], AF.Relu,
                                bias=bv[:, bias_col : bias_col + 1])

                    steps.append(step)
                if li == 2:
                    steps.append(dma_x)
            return steps, z

        ev_off = 0
        yout_off = 0
        g0, cap0 = groups[0]
        cur_steps, cur_z = make_jets(g0, cap0, 0)
        for st in cur_steps:
            st()
        jets_off = g0 * cap0
        for gi, (g, cap) in enumerate(groups):
            assert cap % 8 == 0 and cap <= 512
            prs = pairs_of(g)
            PG = len(prs)
            z = cur_z

            if gi + 1 < len(groups):
                g2, cap2 = groups[gi + 1]
                nxt_steps, nxt_z = make_jets(g2, cap2, jets_off)
                jets_off += g2 * cap2
            else:
                nxt_steps, nxt_z = [], None
            squeue = list(nxt_steps)

            # ---- pairs: SC-slice chunks; y3 streams to DRAM per chunk.
            SC = max(1, 2048 // cap)
            PCH = SC * cap
            y3_i = [0]
            chunk_starts = list(range(0, PG, SC))
            for ci, p0 in enumerate(chunk_starts):
                k = min(SC, PG - p0)
                w = k * cap
                y1 = scr.tile([H, PCH], bf16, tag="y1")
                s = 0
                while s < k:
                    i = prs[p0 + s][0]
                    r = 1
                    while s + r < k and prs[p0 + s + r][0] == i:
                        r += 1
                    j0 = prs[p0 + s][1]
                    nc.vector.tensor_tensor(
                        r3(y1[:, s * cap : (s + r) * cap], r),
                        r3(z[:, i * cap : (i + 1) * cap], 1).broadcast_to(
                            [H, r, cap]),
                        r3(z[:, j0 * cap : (j0 + r) * cap], r),
                        ALU.add)
                    s += r
                nc.vector.tensor_scalar(y1[:, :w], y1[:, :w], bv[:, 3:4],
                                        0.0, ALU.add, ALU.max)
                y2 = scr.tile([H, PCH], bf16, tag="y2")
                for n0 in range(0, w, 1024):
                    cw = min(1024, w - n0)
                    ps = mm.tile([H, 1024], f32, tag="mm")
                    for s0 in range(0, cw, 512):
                        sw = min(512, cw - s0)
                        nc.tensor.matmul(ps[:, s0 : s0 + sw], w4t[:],
                                         y1[:, n0 + s0 : n0 + s0 + sw],
                                         start=True, stop=True)
                    nc.scalar.activation(y2[:, n0 : n0 + cw], ps[:, :cw],
                                         AF.Relu, bias=bv[:, 4:5])
                y3 = scr.tile([H, PCH], bf16, tag="y3")
                for n0 in range(0, w, 1024):
                    cw = min(1024, w - n0)
                    ps = mm.tile([H, 1024], f32, tag="mm")
                    for s0 in range(0, cw, 512):
                        sw = min(512, cw - s0)
                        nc.tensor.matmul(ps[:, s0 : s0 + sw], w5t[:],
                                         y2[:, n0 + s0 : n0 + s0 + sw],
                                         start=True, stop=True)
                    dst = y3[:, n0 : n0 + cw]
                    if y3_i[0] % 3 == 2:
                        nc.vector.tensor_scalar(dst, ps[:, :cw], bv[:, 5:6],
                                                0.0, ALU.add, ALU.max)
                    else:
                        nc.scalar.activation(dst, ps[:, :cw], AF.Relu,
                                             bias=bv[:, 5:6])
                    y3_i[0] += 1
                nc.sync.dma_start(
                    yout_d.ap()[:, yout_off : yout_off + w], y3[:, :w])
                yout_off += w
                # Pace the next group's jets steps across this group's
                # remaining chunks (leftovers drain after the loop).
                rem = len(chunk_starts) - 1 - ci
                if rem > 0:
                    npop = -(-len(squeue) // (rem + 1))
                    for _ in range(min(npop, len(squeue))):
                        squeue.pop(0)()

            for st in squeue:
                st()

            cur_steps, cur_z = nxt_steps, nxt_z
            ev_off += cap

    nc.compile()
    return nc


# ---------------- host-side math ----------------

BN_EPS = 1e-3


def fold_params(inp):
    """Fold normalization + BN into per-layer (W, b). All numpy fp32."""
    mean_j = np.asarray(inp["mean_jets"], np.float32)
    std_j = np.asarray(inp["std_jets"], np.float32)
    w1f = np.asarray(inp["w1_first"], np.float32)
    w1r = np.asarray(inp["w1_rest"], np.float32)
    bn1 = np.asarray(inp["bn1"], np.float32)  # [3,4,H]: gamma, beta, mean, var
    w2f = np.asarray(inp["w2_first"], np.float32)
    w2r = np.asarray(inp["w2_rest"], np.float32)
    bn2 = np.asarray(inp["bn2"], np.float32)

    def bn_sb(row):
        gm, bt, mu, vv = row[0], row[1], row[2], row[3]
        s = gm / np.sqrt(vv + BN_EPS)
        return s.astype(np.float32), (bt - mu * s).astype(np.float32)

    s11, t11 = bn_sb(bn1[0]); s12, t12 = bn_sb(bn1[1]); s13, t13 = bn_sb(bn1[2])
    s21, t21 = bn_sb(bn2[0]); s22, t22 = bn_sb(bn2[1]); s23, t23 = bn_sb(bn2[2])

    A = w1f / std_j[:, None]
    c = -(mean_j / std_j) @ w1f
    return dict(
        W1=A * s11[None, :], b1=c * s11 + t11,
        W2=w1r[0] * s12[None, :], b2=t12,
        W3=w1r[1] * s13[None, :], b3=t13,
        Wz=w2f * s21[None, :], bz=t21,
        W4=w2r[0] * s22[None, :], b4=t22,
        W5=w2r[1] * s23[None, :], b5=t23,
    )


# ---------------- full kernel entry point ----------------

N_CORES = 8

_cache = {}
_TRACE = [False]
_LAST_RESULT = [None]


def _get_program(groups_key):
    if groups_key not in _cache:
        _cache[groups_key] = build_program(list(groups_key))
    return _cache[groups_key]


def _np_dt(dt):
    return mybir.dt.np(dt)


def _plan(n):
    """Returns (groups, slots): groups = [(g, cap)], slots[c][gi] =
    (padded index array, real count) for core c, group gi."""
    gs = []
    idx_by_g = {}
    for g in range(2, 11):
        idx = np.nonzero(n == g)[0]
        if len(idx):
            gs.append(g)
            idx_by_g[g] = idx
    stray = np.nonzero((n < 2) | (n > 10))[0]
    if len(stray):
        if not gs:
            gs.append(2)
            idx_by_g[2] = stray
        else:
            idx_by_g[gs[-1]] = np.concatenate([idx_by_g[gs[-1]], stray])
    # Descending size order: each group's jets chain is emitted inside
    # the previous (bigger) group's pair phase, so it pipelines fully.
    gs = sorted(gs, key=lambda g: -g)
    groups = []
    slots = [[] for _ in range(N_CORES)]
    for g in gs:
        idx = idx_by_g[g]
        per_core = [idx[c::N_CORES] for c in range(N_CORES)]
        mx = max(len(p) for p in per_core)
        cap = max(8, ((mx + 7) // 8) * 8)
        groups.append((g, cap))
        fill = idx[0]
        for c in range(N_CORES):
            p = per_core[c]
            pad = np.full(cap, p[0] if len(p) else fill, dtype=np.int64)
            pad[: len(p)] = p
            slots[c].append((pad, len(p)))
    return groups, slots


def _pack_jets(jets, groups, slots_c):
    cols = []
    for (g, cap), (ids, _cnt) in zip(groups, slots_c):
        ev = jets[ids][:, :g, :]  # [cap, g, 16]
        cols.append(np.ascontiguousarray(ev.transpose(2, 1, 0)).reshape(
            FJ, g * cap))
    return np.concatenate(cols, axis=1).astype(_np_dt(bf16), copy=False)


def kernel(**inputs):
    from concourse.bass_utils import run_bass_kernel_spmd

    jets = np.asarray(inputs["inputs_jets"], dtype=np.float32)
    B = jets.shape[0]
    mask = (jets != 0.0).any(-1)
    n = mask.sum(-1).astype(np.int64)
    # compact valid jets to the front (no-op for the standard generator)
    if not np.array_equal(mask, np.arange(jets.shape[1])[None, :] < n[:, None]):
        order = np.argsort(~mask, axis=1, kind="stable")
        jets = np.take_along_axis(jets, order[:, :, None], axis=1)

    P = fold_params(inputs)
    groups, slots = _plan(n)
    nc = _get_program(tuple(groups))

    bvec = np.zeros((H, 8), np.float32)
    for i, k in enumerate(["b1", "b2", "b3", "bz", "b4", "b5"]):
        bvec[:, i] = P[k]
    bnp = _np_dt(bf16)
    common = {
        "w1": P["W1"].astype(bnp), "w2": P["W2"].astype(bnp),
        "w3": P["W3"].astype(bnp), "wz": P["Wz"].astype(bnp),
        "w4": P["W4"].astype(bnp), "w5": P["W5"].astype(bnp),
        "bvec": bvec,
    }
    in_maps = []
    for c in range(N_CORES):
        m = dict(common)
        m["jets"] = _pack_jets(jets, groups, slots[c])
        in_maps.append(m)

    res = run_bass_kernel_spmd(nc, in_maps, core_ids=list(range(N_CORES)),
                               trace=_TRACE[0])
    _LAST_RESULT[0] = res

    agg_x = np.empty((B, 4 * H), np.float32)
    agg_y = np.empty((B, 4 * H), np.float32)
    for c in range(N_CORES):
        ox = np.asarray(res.results[c]["xout"])  # [H, JC] bf16
        oy = np.asarray(res.results[c]["yout"])  # [H, YC] bf16
        joff = 0
        yoff = 0
        for (g, cap), (ids, cnt) in zip(groups, slots[c]):
            PGg = g * (g - 1) // 2
            ii = ids[:cnt]
            xb = ox[:, joff : joff + g * cap].astype(np.float32)
            xb = xb.reshape(H, g, cap)[:, :, :cnt]
            sx = xb.sum(1).T
            mx = xb.max(1).T
            qx = (xb * xb).sum(1).T
            mean_x = sx / g
            agg_x[ii] = np.concatenate(
                [sx, mx, mean_x, qx / g - mean_x * mean_x], axis=1)
            yb = oy[:, yoff : yoff + PGg * cap].astype(np.float32)
            yb = yb.reshape(H, PGg, cap)[:, :, :cnt]
            sy = yb.sum(1).T
            my = yb.max(1).T
            qy = (yb * yb).sum(1).T
            mean_y = sy / PGg
            agg_y[ii] = np.concatenate(
                [sy, my, mean_y, qy / PGg - mean_y * mean_y], axis=1)
            joff += g * cap
            yoff += PGg * cap
    return agg_x, agg_y


# revision 61
# speedup vs baseline: 1.1932x; 1.0340x over previous
# DeepSet Trainium2 kernel.
#
# Strategy: events are sorted by jet-count n (2..10) on the host and
# round-robin sharded across 8 cores into per-group slots of capacity cap_g
# (multiple of 8, exact-packed). Within a group every event has exactly n=g
# valid jets, so all masks, pair structures and aggregation counts are
# compile-time constants.
#
# Math folding (host, O(params)):
#   every Dense+BN+relu block becomes relu(h @ W' + b') with W', b' folded.
#   MLP2 layer 1 uses the z-trick: y1 = relu(z_i + z_j + t) with z = x @ Wz'.
#   t is folded into the y1 relu pass (tensor_scalar add-bias + max0).
#
# Device layout: feature-major [H=128 partitions, columns = slice*cap + b]
# per group, all activations bf16 (PE: 1 col/cycle), PSUM f32.
# The device computes ONLY the MLP chains:
#   jets:  x1 = relu(W1 jt + b1); x2 = relu(W2 x1 + b2); x = relu(W3 x2 + b3)
#          z = Wz x (plain copy evac)
#   pairs: y1 = relu(z_i + z_j + t) (DVE broadcast-add + tensor_scalar 4x)
#          y2 = relu(W4 y1 + b4); y3 = relu(W5 y2 + b5)
# and streams x and y3 (bf16) to DRAM. All aggregations (sum/max/mean/var
# over jets and pairs) happen on the HOST in f32 — the DMA engines were
# ~10% busy while Vector/Scalar were the kernel bottleneck, so shipping
# raw activations beats computing reductions on-device.
#
# Engine split (measured rates, ns/col of 128 rows):
#   Scalar  (~1.05): x1/x2/x/z PSUM evacs, y2 evac, y3 evac (2 of 3)
#   Vector  (~0.65 TT / 0.26 TSP-bf16 / 1.05 PSUM): y1 add, y1 relu,
#           y3 evac (1 of 3)
#   PE:     all matmuls (bf16 1 col/cycle, dual-buffered weight loads)
#   GPSIMD: unused — no PSUM access, no max opcode, slow tensor_scalar,
#           and its tensor_tensor traffic degrades Vector ~25% (measured).
#
# The next group's jets layer-chunks are emitted interleaved into the
# current group's pair-chunk loop (groups in descending size order), so
# PE/Scalar advance the next group while Vector drains the current one.
from contextlib import ExitStack

import numpy as np

import concourse.bass as bass
import concourse.bacc as bacc
import concourse.tile as tile
import concourse.mybir as mybir

f32 = mybir.dt.float32
bf16 = mybir.dt.bfloat16
AF = mybir.ActivationFunctionType
ALU = mybir.AluOpType

H = 128
FJ = 16


def pairs_of(g):
    return [(i, j) for i in range(g) for j in range(i + 1, g)]


# y3 PSUM-evac engine pattern (s=Scalar, v=Vector), tuned from traces.
Y3_PAT = ("s", "v", "v", "s", "v")


def build_program(groups):
    """groups: list of (g, cap) with cap a multiple of 8, cap <= 512."""
    JC = sum(g * cap for g, cap in groups)
    YC = sum((g * (g - 1) // 2) * cap for g, cap in groups)

    nc = bacc.Bacc("TRN2", target_bir_lowering=False, debug=False)

    jets_d = nc.dram_tensor("jets", [FJ, JC], bf16, kind="ExternalInput")
    w1_d = nc.dram_tensor("w1", [FJ, H], bf16, kind="ExternalInput")
    w2_d = nc.dram_tensor("w2", [H, H], bf16, kind="ExternalInput")
    w3_d = nc.dram_tensor("w3", [H, H], bf16, kind="ExternalInput")
    wz_d = nc.dram_tensor("wz", [H, H], bf16, kind="ExternalInput")
    w4_d = nc.dram_tensor("w4", [H, H], bf16, kind="ExternalInput")
    w5_d = nc.dram_tensor("w5", [H, H], bf16, kind="ExternalInput")
    # bias vector cols: 0..5 = b1, b2, b3, t(=bz), b4, b5
    bv_d = nc.dram_tensor("bvec", [H, 8], f32, kind="ExternalInput")
    xout_d = nc.dram_tensor("xout", [H, JC], bf16, kind="ExternalOutput")
    yout_d = nc.dram_tensor("yout", [H, YC], bf16, kind="ExternalOutput")

    with tile.TileContext(nc) as tc, ExitStack() as ctx:
        consts = ctx.enter_context(tc.tile_pool(name="consts", bufs=1))
        jin = ctx.enter_context(tc.tile_pool(name="jin", bufs=2))
        x12 = ctx.enter_context(tc.tile_pool(name="x12", bufs=2))
        bigx = ctx.enter_context(tc.tile_pool(name="bigx", bufs=2))
        scr = ctx.enter_context(tc.tile_pool(name="scr", bufs=3))
        mm = ctx.enter_context(tc.tile_pool(name="mm", bufs=4, space="PSUM"))

        def const_tile(name, dram, shape, dt):
            t = consts.tile(shape, dt, tag=name)
            nc.sync.dma_start(t[:], dram.ap())
            return t

        w1t = const_tile("w1", w1_d, [FJ, H], bf16)
        w2t = const_tile("w2", w2_d, [H, H], bf16)
        w3t = const_tile("w3", w3_d, [H, H], bf16)
        wzt = const_tile("wz", wz_d, [H, H], bf16)
        w4t = const_tile("w4", w4_d, [H, H], bf16)
        w5t = const_tile("w5", w5_d, [H, H], bf16)
        bv = const_tile("bv", bv_d, [H, 8], f32)

        def r3(ap, k):
            return ap.rearrange("p (k c) -> p k c", k=k)

        # ---- jets side: 4 layers as a list of emission steps (one
        # 1024-col layer-chunk each). Steps of group g+1 are interleaved
        # into group g's pair-chunk loop.
        def make_jets(g, cap, jets_off):
            JCg = g * cap
            jt = jin.tile([FJ, JCg], bf16, tag="jt")
            x1 = x12.tile([H, JCg], bf16, tag="x1")
            x2 = x12.tile([H, JCg], bf16, tag="x2")
            x = bigx.tile([H, JCg], bf16, tag="x")
            z = bigx.tile([H, JCg], bf16, tag="z")

            def dma_in(jt=jt, jets_off=jets_off, JCg=JCg):
                nc.sync.dma_start(jt[:],
                                  jets_d.ap()[:, jets_off : jets_off + JCg])

            def dma_x(x=x, jets_off=jets_off, JCg=JCg):
                nc.sync.dma_start(
                    xout_d.ap()[:, jets_off : jets_off + JCg], x[:])

            steps = [dma_in]
            plan = [(x1, w1t, jt, 0), (x2, w2t, x1, 1),
                    (x, w3t, x2, 2), (z, wzt, x, None)]
            for li, (dst, wt, src, bias_col) in enumerate(plan):
                for c0 in range(0, JCg, 1024):
                    w = min(1024, JCg - c0)

                    def step(dst=dst, wt=wt, src=src, bias_col=bias_col,
                             c0=c0, w=w, li=li):
                        ps = mm.tile([H, 1024], f32, tag="mm")
                        for s0 in range(0, w, 512):
                            sw = min(512, w - s0)
                            nc.tensor.matmul(ps[:, s0 : s0 + sw], wt[:],
                                             src[:, c0 + s0 : c0 + s0 + sw],
                                             start=True, stop=True)
                        # L1/L2 evac on Scalar; L3 (x) and Lz (z) on
                        # Vector — balances the two evac engines.
                        if bias_col is None:
                            nc.vector.tensor_copy(dst[:, c0 : c0 + w],
                                                  ps[:, :w])
                        elif li == 2:
                            nc.vector.tensor_scalar(
                                dst[:, c0 : c0 + w], ps[:, :w],
                                bv[:, bias_col : bias_col + 1], 0.0,
                                ALU.add, ALU.max)
                        else:
                            nc.scalar.activation(
                                dst[:, c0 : c0 + w], ps[:, :w], AF.Relu,
                                bias=bv[:, bias_col : bias_col + 1])

                    steps.append(step)
                if li == 2:
                    steps.append(dma_x)
            return steps, z

        ev_off = 0
        yout_off = 0
        g0, cap0 = groups[0]
        cur_steps, cur_z = make_jets(g0, cap0, 0)
        for st in cur_steps:
            st()
        jets_off = g0 * cap0
        for gi, (g, cap) in enumerate(groups):
            assert cap % 8 == 0 and cap <= 512
            prs = pairs_of(g)
            PG = len(prs)
            z = cur_z

            if gi + 1 < len(groups):
                g2, cap2 = groups[gi + 1]
                nxt_steps, nxt_z = make_jets(g2, cap2, jets_off)
                jets_off += g2 * cap2
            else:
                nxt_steps, nxt_z = [], None
            squeue = list(nxt_steps)

            # ---- pairs: SC-slice chunks; y3 streams to DRAM per chunk.
            SC = max(1, 2048 // cap)
            PCH = SC * cap
            y3_i = [0]
            chunk_starts = list(range(0, PG, SC))
            for ci, p0 in enumerate(chunk_starts):
                k = min(SC, PG - p0)
                w = k * cap
                y1 = scr.tile([H, PCH], bf16, tag="y1")
                s = 0
                while s < k:
                    i = prs[p0 + s][0]
                    r = 1
                    while s + r < k and prs[p0 + s + r][0] == i:
                        r += 1
                    j0 = prs[p0 + s][1]
                    nc.vector.tensor_tensor(
                        r3(y1[:, s * cap : (s + r) * cap], r),
                        r3(z[:, i * cap : (i + 1) * cap], 1).broadcast_to(
                            [H, r, cap]),
                        r3(z[:, j0 * cap : (j0 + r) * cap], r),
                        ALU.add)
                    s += r
                nc.vector.tensor_scalar(y1[:, :w], y1[:, :w], bv[:, 3:4],
                                        0.0, ALU.add, ALU.max)
                y2 = scr.tile([H, PCH], bf16, tag="y2")
                for n0 in range(0, w, 1024):
                    cw = min(1024, w - n0)
                    ps = mm.tile([H, 1024], f32, tag="mm")
                    for s0 in range(0, cw, 512):
                        sw = min(512, cw - s0)
                        nc.tensor.matmul(ps[:, s0 : s0 + sw], w4t[:],
                                         y1[:, n0 + s0 : n0 + s0 + sw],
                                         start=True, stop=True)
                    nc.scalar.activation(y2[:, n0 : n0 + cw], ps[:, :cw],
                                         AF.Relu, bias=bv[:, 4:5])
                y3 = scr.tile([H, PCH], bf16, tag="y3")
                for n0 in range(0, w, 1024):
                    cw = min(1024, w - n0)
                    ps = mm.tile([H, 1024], f32, tag="mm")
                    for s0 in range(0, cw, 512):
                        sw = min(512, cw - s0)
                        nc.tensor.matmul(ps[:, s0 : s0 + sw], w5t[:],
                                         y2[:, n0 + s0 : n0 + s0 + sw],
                                         start=True, stop=True)
                    dst = y3[:, n0 : n0 + cw]
                    if Y3_PAT[y3_i[0] % len(Y3_PAT)] == "v":
                        nc.vector.tensor_scalar(dst, ps[:, :cw], bv[:, 5:6],
                                                0.0, ALU.add, ALU.max)
                    else:
                        nc.scalar.activation(dst, ps[:, :cw], AF.Relu,
                                             bias=bv[:, 5:6])
                    y3_i[0] += 1
                nc.sync.dma_start(
                    yout_d.ap()[:, yout_off : yout_off + w], y3[:, :w])
                yout_off += w
                # Pace the next group's jets steps across this group's
                # remaining chunks (leftovers drain after the loop).
                rem = len(chunk_starts) - 1 - ci
                if rem > 0:
                    npop = -(-len(squeue) // (rem + 1))
                    for _ in range(min(npop, len(squeue))):
                        squeue.pop(0)()

            for st in squeue:
                st()

            cur_steps, cur_z = nxt_steps, nxt_z
            ev_off += cap

    nc.compile()
    return nc


# ---------------- host-side math ----------------

BN_EPS = 1e-3


def fold_params(inp):
    """Fold normalization + BN into per-layer (W, b). All numpy fp32."""
    mean_j = np.asarray(inp["mean_jets"], np.float32)
    std_j = np.asarray(inp["std_jets"], np.float32)
    w1f = np.asarray(inp["w1_first"], np.float32)
    w1r = np.asarray(inp["w1_rest"], np.float32)
    bn1 = np.asarray(inp["bn1"], np.float32)  # [3,4,H]: gamma, beta, mean, var
    w2f = np.asarray(inp["w2_first"], np.float32)
    w2r = np.asarray(inp["w2_rest"], np.float32)
    bn2 = np.asarray(inp["bn2"], np.float32)

    def bn_sb(row):
        gm, bt, mu, vv = row[0], row[1], row[2], row[3]
        s = gm / np.sqrt(vv + BN_EPS)
        return s.astype(np.float32), (bt - mu * s).astype(np.float32)

    s11, t11 = bn_sb(bn1[0]); s12, t12 = bn_sb(bn1[1]); s13, t13 = bn_sb(bn1[2])
    s21, t21 = bn_sb(bn2[0]); s22, t22 = bn_sb(bn2[1]); s23, t23 = bn_sb(bn2[2])

    A = w1f / std_j[:, None]
    c = -(mean_j / std_j) @ w1f
    return dict(
        W1=A * s11[None, :], b1=c * s11 + t11,
        W2=w1r[0] * s12[None, :], b2=t12,
        W3=w1r[1] * s13[None, :], b3=t13,
        Wz=w2f * s21[None, :], bz=t21,
        W4=w2r[0] * s22[None, :], b4=t22,
        W5=w2r[1] * s23[None, :], b5=t23,
    )


# ---------------- full kernel entry point ----------------

N_CORES = 8

_cache = {}
_TRACE = [False]
_LAST_RESULT = [None]


def _get_program(groups_key):
    if groups_key not in _cache:
        _cache[groups_key] = build_program(list(groups_key))
    return _cache[groups_key]


def _np_dt(dt):
    return mybir.dt.np(dt)


def _plan(n):
    """Returns (groups, slots): groups = [(g, cap)], slots[c][gi] =
    (padded index array, real count) for core c, group gi."""
    gs = []
    idx_by_g = {}
    for g in range(2, 11):
        idx = np.nonzero(n == g)[0]
        if len(idx):
            gs.append(g)
            idx_by_g[g] = idx
    stray = np.nonzero((n < 2) | (n > 10))[0]
    if len(stray):
        if not gs:
            gs.append(2)
            idx_by_g[2] = stray
        else:
            idx_by_g[gs[-1]] = np.concatenate([idx_by_g[gs[-1]], stray])
    # Descending size order: each group's jets chain is emitted inside
    # the previous (bigger) group's pair phase, so it pipelines fully.
    gs = sorted(gs, key=lambda g: -g)
    groups = []
    slots = [[] for _ in range(N_CORES)]
    for g in gs:
        idx = idx_by_g[g]
        per_core = [idx[c::N_CORES] for c in range(N_CORES)]
        mx = max(len(p) for p in per_core)
        cap = max(8, ((mx + 7) // 8) * 8)
        groups.append((g, cap))
        fill = idx[0]
        for c in range(N_CORES):
            p = per_core[c]
            pad = np.full(cap, p[0] if len(p) else fill, dtype=np.int64)
            pad[: len(p)] = p
            slots[c].append((pad, len(p)))
    return groups, slots


def _pack_jets(jets, groups, slots_c):
    cols = []
    for (g, cap), (ids, _cnt) in zip(groups, slots_c):
        ev = jets[ids][:, :g, :]  # [cap, g, 16]
        cols.append(np.ascontiguousarray(ev.transpose(2, 1, 0)).reshape(
            FJ, g * cap))
    return np.concatenate(cols, axis=1).astype(_np_dt(bf16), copy=False)


def kernel(**inputs):
    from concourse.bass_utils import run_bass_kernel_spmd

    jets = np.asarray(inputs["inputs_jets"], dtype=np.float32)
    B = jets.shape[0]
    mask = (jets != 0.0).any(-1)
    n = mask.sum(-1).astype(np.int64)
    # compact valid jets to the front (no-op for the standard generator)
    if not np.array_equal(mask, np.arange(jets.shape[1])[None, :] < n[:, None]):
        order = np.argsort(~mask, axis=1, kind="stable")
        jets = np.take_along_axis(jets, order[:, :, None], axis=1)

    P = fold_params(inputs)
    groups, slots = _plan(n)
    nc = _get_program(tuple(groups))

    bvec = np.zeros((H, 8), np.float32)
    for i, k in enumerate(["b1", "b2", "b3", "bz", "b4", "b5"]):
        bvec[:, i] = P[k]
    bnp = _np_dt(bf16)
    common = {
        "w1": P["W1"].astype(bnp), "w2": P["W2"].astype(bnp),
        "w3": P["W3"].astype(bnp), "wz": P["Wz"].astype(bnp),
        "w4": P["W4"].astype(bnp), "w5": P["W5"].astype(bnp),
        "bvec": bvec,
    }
    in_maps = []
    for c in range(N_CORES):
        m = dict(common)
        m["jets"] = _pack_jets(jets, groups, slots[c])
        in_maps.append(m)

    res = run_bass_kernel_spmd(nc, in_maps, core_ids=list(range(N_CORES)),
                               trace=_TRACE[0])
    _LAST_RESULT[0] = res

    agg_x = np.empty((B, 4 * H), np.float32)
    agg_y = np.empty((B, 4 * H), np.float32)
    for c in range(N_CORES):
        ox = np.asarray(res.results[c]["xout"])  # [H, JC] bf16
        oy = np.asarray(res.results[c]["yout"])  # [H, YC] bf16
        joff = 0
        yoff = 0
        for (g, cap), (ids, cnt) in zip(groups, slots[c]):
            PGg = g * (g - 1) // 2
            ii = ids[:cnt]
            xb = ox[:, joff : joff + g * cap].astype(np.float32)
            xb = xb.reshape(H, g, cap)[:, :, :cnt]
            sx = xb.sum(1).T
            mx = xb.max(1).T
            qx = (xb * xb).sum(1).T
            mean_x = sx / g
            agg_x[ii] = np.concatenate(
                [sx, mx, mean_x, qx / g - mean_x * mean_x], axis=1)
            yb = oy[:, yoff : yoff + PGg * cap].astype(np.float32)
            yb = yb.reshape(H, PGg, cap)[:, :, :cnt]
            sy = yb.sum(1).T
            my = yb.max(1).T
            qy = (yb * yb).sum(1).T
            mean_y = sy / PGg
            agg_y[ii] = np.concatenate(
                [sy, my, mean_y, qy / PGg - mean_y * mean_y], axis=1)
            joff += g * cap
            yoff += PGg * cap
    return agg_x, agg_y


# revision 63
# speedup vs baseline: 1.5287x; 1.2812x over previous
# DeepSet Trainium2 kernel.
#
# Strategy: events are sorted by jet-count n (2..10) on the host and
# round-robin sharded across 8 cores into per-group slots of capacity cap_g
# (multiple of 8, exact-packed). Within a group every event has exactly n=g
# valid jets, so all masks, pair structures and aggregation counts are
# compile-time constants.
#
# Math folding (host, O(params)):
#   every Dense+BN+relu block becomes relu(h @ W' + b') with W', b' folded.
#   MLP2 layer 1 uses the z-trick: y1 = relu(z_i + z_j + t) with z = x @ Wz'.
#   t is folded into the y1 relu pass (tensor_scalar add-bias + max0).
#
# Device layout: feature-major [H=128 partitions, columns = slice*cap + b]
# per group, all activations bf16 (PE: 1 col/cycle), PSUM f32.
# The device computes ONLY the MLP chains:
#   jets:  x1 = relu(W1 jt + b1); x2 = relu(W2 x1 + b2); x = relu(W3 x2 + b3)
#          z = Wz x (plain copy evac)
#   pairs: y1 = relu(z_i + z_j + t) (DVE broadcast-add + tensor_scalar 4x)
#          y2 = relu(W4 y1 + b4); y3 = relu(W5 y2 + b5)
# and streams x and y3 (bf16) to DRAM. All aggregations (sum/max/mean/var
# over jets and pairs) happen on the HOST in f32 — the DMA engines were
# ~10% busy while Vector/Scalar were the kernel bottleneck, so shipping
# raw activations beats computing reductions on-device.
#
# Engine split (measured rates, ns/col of 128 rows):
#   Scalar  (~1.05): x1/x2/x/z PSUM evacs, y2 evac, y3 evac (2 of 3)
#   Vector  (~0.65 TT / 0.26 TSP-bf16 / 1.05 PSUM): y1 add, y1 relu,
#           y3 evac (1 of 3)
#   PE:     all matmuls (bf16 1 col/cycle, dual-buffered weight loads)
#   GPSIMD: unused — no PSUM access, no max opcode, slow tensor_scalar,
#           and its tensor_tensor traffic degrades Vector ~25% (measured).
#
# The next group's jets layer-chunks are emitted interleaved into the
# current group's pair-chunk loop (groups in descending size order), so
# PE/Scalar advance the next group while Vector drains the current one.
from contextlib import ExitStack

import numpy as np

import concourse.bass as bass
import concourse.bacc as bacc
import concourse.tile as tile
import concourse.mybir as mybir

f32 = mybir.dt.float32
bf16 = mybir.dt.bfloat16
AF = mybir.ActivationFunctionType
ALU = mybir.AluOpType

H = 128
FJ = 16


def pairs_of(g):
    return [(i, j) for i in range(g) for j in range(i + 1, g)]


# y3 PSUM-evac engine pattern (s=Scalar, v=Vector), tuned from traces.
Y3_PAT = ("s", "v", "v", "s", "v")


def build_program(groups):
    """groups: list of (g, cap) with cap a multiple of 8, cap <= 512."""
    JC = sum(g * cap for g, cap in groups)
    YC = sum((g * (g - 1) // 2) * cap for g, cap in groups)

    nc = bacc.Bacc("TRN2", target_bir_lowering=False, debug=False)

    jets_d = nc.dram_tensor("jets", [FJ, JC], bf16, kind="ExternalInput")
    w1_d = nc.dram_tensor("w1", [FJ, H], bf16, kind="ExternalInput")
    w2_d = nc.dram_tensor("w2", [H, H], bf16, kind="ExternalInput")
    w3_d = nc.dram_tensor("w3", [H, H], bf16, kind="ExternalInput")
    wz_d = nc.dram_tensor("wz", [H, H], bf16, kind="ExternalInput")
    w4_d = nc.dram_tensor("w4", [H, H], bf16, kind="ExternalInput")
    w5_d = nc.dram_tensor("w5", [H, H], bf16, kind="ExternalInput")
    # bias vector cols: 0..5 = b1, b2, b3, t(=bz), b4, b5
    bv_d = nc.dram_tensor("bvec", [H, 8], f32, kind="ExternalInput")
    xout_d = nc.dram_tensor("xout", [H, JC], bf16, kind="ExternalOutput")
    yout_d = nc.dram_tensor("yout", [H, YC], bf16, kind="ExternalOutput")

    with tile.TileContext(nc) as tc, ExitStack() as ctx:
        consts = ctx.enter_context(tc.tile_pool(name="consts", bufs=1))
        big = ctx.enter_context(tc.tile_pool(name="big", bufs=1))
        scr = ctx.enter_context(tc.tile_pool(name="scr", bufs=2))
        mm = ctx.enter_context(tc.tile_pool(name="mm", bufs=4, space="PSUM"))

        def const_tile(name, dram, shape, dt):
            t = consts.tile(shape, dt, tag=name)
            nc.sync.dma_start(t[:], dram.ap())
            return t

        w1t = const_tile("w1", w1_d, [FJ, H], bf16)
        w2t = const_tile("w2", w2_d, [H, H], bf16)
        w3t = const_tile("w3", w3_d, [H, H], bf16)
        wzt = const_tile("wz", wz_d, [H, H], bf16)
        w4t = const_tile("w4", w4_d, [H, H], bf16)
        w5t = const_tile("w5", w5_d, [H, H], bf16)
        bv = const_tile("bv", bv_d, [H, 8], f32)

        def r3(ap, k):
            return ap.rearrange("p (k c) -> p k c", k=k)

        # ---- phase 1: jets MLP over the whole [*, JC] width, no group
        # boundaries. Layer-major: every chunk of a layer only depends on
        # the same chunk of the previous layer (emitted a full layer
        # earlier), so all three engines pipeline freely.
        jt = big.tile([FJ, JC], bf16, tag="jt")
        for c0 in range(0, JC, 2048):
            w = min(2048, JC - c0)
            nc.sync.dma_start(jt[:, c0 : c0 + w],
                              jets_d.ap()[:, c0 : c0 + w])
        x1 = big.tile([H, JC], bf16, tag="x1")
        x2 = big.tile([H, JC], bf16, tag="x2")
        x = big.tile([H, JC], bf16, tag="x")
        z = big.tile([H, JC], bf16, tag="z")
        plan = [(x1, w1t, jt, 0, "s"), (x2, w2t, x1, 1, "s"),
                (x, w3t, x2, 2, "v"), (z, wzt, x, None, "v")]
        for li, (dst, wt, src, bias_col, eng) in enumerate(plan):
            for c0 in range(0, JC, 1024):
                w = min(1024, JC - c0)
                ps = mm.tile([H, 1024], f32, tag="mm")
                for s0 in range(0, w, 512):
                    sw = min(512, w - s0)
                    nc.tensor.matmul(ps[:, s0 : s0 + sw], wt[:],
                                     src[:, c0 + s0 : c0 + s0 + sw],
                                     start=True, stop=True)
                # L1/L2 evacs on Scalar; L3 (x) and Lz (z) on Vector.
                if bias_col is None:
                    nc.vector.tensor_copy(dst[:, c0 : c0 + w], ps[:, :w])
                elif eng == "v":
                    nc.vector.tensor_scalar(
                        dst[:, c0 : c0 + w], ps[:, :w],
                        bv[:, bias_col : bias_col + 1], 0.0,
                        ALU.add, ALU.max)
                else:
                    nc.scalar.activation(
                        dst[:, c0 : c0 + w], ps[:, :w], AF.Relu,
                        bias=bv[:, bias_col : bias_col + 1])
                if li == 2:
                    nc.sync.dma_start(xout_d.ap()[:, c0 : c0 + w],
                                      dst[:, c0 : c0 + w])

        # ---- phase 2: all pair chunks stream through y1 -> y2 -> y3 ->
        # DRAM. Every chunk's z is ready, so chunks are fully independent
        # and pipeline across V (y1), PE (matmuls), S/V (evacs), DMA.
        # y3 matmuls run one chunk behind y2 (software pipeline) so PE
        # never waits on the y2 evac of the chunk it just produced.
        yout_off = 0
        y3_i = [0]
        pend = [None]  # (y2_tile, w, yout_off)

        def emit_y3(y2, w, yoff):
            y3 = scr.tile([H, 4096], bf16, tag="y3")
            for n0 in range(0, w, 1024):
                cw = min(1024, w - n0)
                ps = mm.tile([H, 1024], f32, tag="mm")
                for s0 in range(0, cw, 512):
                    sw = min(512, cw - s0)
                    nc.tensor.matmul(ps[:, s0 : s0 + sw], w5t[:],
                                     y2[:, n0 + s0 : n0 + s0 + sw],
                                     start=True, stop=True)
                dst = y3[:, n0 : n0 + cw]
                if Y3_PAT[y3_i[0] % len(Y3_PAT)] == "v":
                    nc.vector.tensor_scalar(dst, ps[:, :cw], bv[:, 5:6],
                                            0.0, ALU.add, ALU.max)
                else:
                    nc.scalar.activation(dst, ps[:, :cw], AF.Relu,
                                         bias=bv[:, 5:6])
                y3_i[0] += 1
            nc.sync.dma_start(yout_d.ap()[:, yoff : yoff + w], y3[:, :w])

        jets_off = 0
        for g, cap in groups:
            assert cap % 8 == 0 and cap <= 512
            prs = pairs_of(g)
            PG = len(prs)
            SC = max(1, 4096 // cap)
            for p0 in range(0, PG, SC):
                k = min(SC, PG - p0)
                w = k * cap
                y1 = scr.tile([H, 4096], bf16, tag="y1")
                s = 0
                while s < k:
                    i = prs[p0 + s][0]
                    r = 1
                    while s + r < k and prs[p0 + s + r][0] == i:
                        r += 1
                    j0 = prs[p0 + s][1]
                    zb = jets_off
                    nc.vector.tensor_tensor(
                        r3(y1[:, s * cap : (s + r) * cap], r),
                        r3(z[:, zb + i * cap : zb + (i + 1) * cap],
                           1).broadcast_to([H, r, cap]),
                        r3(z[:, zb + j0 * cap : zb + (j0 + r) * cap], r),
                        ALU.add)
                    s += r
                nc.vector.tensor_scalar(y1[:, :w], y1[:, :w], bv[:, 3:4],
                                        0.0, ALU.add, ALU.max)
                y2 = scr.tile([H, 4096], bf16, tag="y2")
                for n0 in range(0, w, 1024):
                    cw = min(1024, w - n0)
                    ps = mm.tile([H, 1024], f32, tag="mm")
                    for s0 in range(0, cw, 512):
                        sw = min(512, cw - s0)
                        nc.tensor.matmul(ps[:, s0 : s0 + sw], w4t[:],
                                         y1[:, n0 + s0 : n0 + s0 + sw],
                                         start=True, stop=True)
                    nc.scalar.activation(y2[:, n0 : n0 + cw], ps[:, :cw],
                                         AF.Relu, bias=bv[:, 4:5])
                if pend[0] is not None:
                    emit_y3(*pend[0])
                pend[0] = (y2, w, yout_off)
                yout_off += w
            jets_off += g * cap
        emit_y3(*pend[0])

    nc.compile()
    return nc


# ---------------- host-side math ----------------

BN_EPS = 1e-3


def fold_params(inp):
    """Fold normalization + BN into per-layer (W, b). All numpy fp32."""
    mean_j = np.asarray(inp["mean_jets"], np.float32)
    std_j = np.asarray(inp["std_jets"], np.float32)
    w1f = np.asarray(inp["w1_first"], np.float32)
    w1r = np.asarray(inp["w1_rest"], np.float32)
    bn1 = np.asarray(inp["bn1"], np.float32)  # [3,4,H]: gamma, beta, mean, var
    w2f = np.asarray(inp["w2_first"], np.float32)
    w2r = np.asarray(inp["w2_rest"], np.float32)
    bn2 = np.asarray(inp["bn2"], np.float32)

    def bn_sb(row):
        gm, bt, mu, vv = row[0], row[1], row[2], row[3]
        s = gm / np.sqrt(vv + BN_EPS)
        return s.astype(np.float32), (bt - mu * s).astype(np.float32)

    s11, t11 = bn_sb(bn1[0]); s12, t12 = bn_sb(bn1[1]); s13, t13 = bn_sb(bn1[2])
    s21, t21 = bn_sb(bn2[0]); s22, t22 = bn_sb(bn2[1]); s23, t23 = bn_sb(bn2[2])

    A = w1f / std_j[:, None]
    c = -(mean_j / std_j) @ w1f
    return dict(
        W1=A * s11[None, :], b1=c * s11 + t11,
        W2=w1r[0] * s12[None, :], b2=t12,
        W3=w1r[1] * s13[None, :], b3=t13,
        Wz=w2f * s21[None, :], bz=t21,
        W4=w2r[0] * s22[None, :], b4=t22,
        W5=w2r[1] * s23[None, :], b5=t23,
    )


# ---------------- full kernel entry point ----------------

N_CORES = 8

_cache = {}
_TRACE = [False]
_LAST_RESULT = [None]


def _get_program(groups_key):
    if groups_key not in _cache:
        _cache[groups_key] = build_program(list(groups_key))
    return _cache[groups_key]


def _np_dt(dt):
    return mybir.dt.np(dt)


def _plan(n):
    """Returns (groups, slots): groups = [(g, cap)], slots[c][gi] =
    (padded index array, real count) for core c, group gi."""
    gs = []
    idx_by_g = {}
    for g in range(2, 11):
        idx = np.nonzero(n == g)[0]
        if len(idx):
            gs.append(g)
            idx_by_g[g] = idx
    stray = np.nonzero((n < 2) | (n > 10))[0]
    if len(stray):
        if not gs:
            gs.append(2)
            idx_by_g[2] = stray
        else:
            idx_by_g[gs[-1]] = np.concatenate([idx_by_g[gs[-1]], stray])
    # Descending size order: each group's jets chain is emitted inside
    # the previous (bigger) group's pair phase, so it pipelines fully.
    gs = sorted(gs, key=lambda g: -g)
    groups = []
    slots = [[] for _ in range(N_CORES)]
    for g in gs:
        idx = idx_by_g[g]
        per_core = [idx[c::N_CORES] for c in range(N_CORES)]
        mx = max(len(p) for p in per_core)
        cap = max(8, ((mx + 7) // 8) * 8)
        groups.append((g, cap))
        fill = idx[0]
        for c in range(N_CORES):
            p = per_core[c]
            pad = np.full(cap, p[0] if len(p) else fill, dtype=np.int64)
            pad[: len(p)] = p
            slots[c].append((pad, len(p)))
    return groups, slots


def _pack_jets(jets, groups, slots_c):
    cols = []
    for (g, cap), (ids, _cnt) in zip(groups, slots_c):
        ev = jets[ids][:, :g, :]  # [cap, g, 16]
        cols.append(np.ascontiguousarray(ev.transpose(2, 1, 0)).reshape(
            FJ, g * cap))
    return np.concatenate(cols, axis=1).astype(_np_dt(bf16), copy=False)


def kernel(**inputs):
    from concourse.bass_utils import run_bass_kernel_spmd

    jets = np.asarray(inputs["inputs_jets"], dtype=np.float32)
    B = jets.shape[0]
    mask = (jets != 0.0).any(-1)
    n = mask.sum(-1).astype(np.int64)
    # compact valid jets to the front (no-op for the standard generator)
    if not np.array_equal(mask, np.arange(jets.shape[1])[None, :] < n[:, None]):
        order = np.argsort(~mask, axis=1, kind="stable")
        jets = np.take_along_axis(jets, order[:, :, None], axis=1)

    P = fold_params(inputs)
    groups, slots = _plan(n)
    nc = _get_program(tuple(groups))

    bvec = np.zeros((H, 8), np.float32)
    for i, k in enumerate(["b1", "b2", "b3", "bz", "b4", "b5"]):
        bvec[:, i] = P[k]
    bnp = _np_dt(bf16)
    common = {
        "w1": P["W1"].astype(bnp), "w2": P["W2"].astype(bnp),
        "w3": P["W3"].astype(bnp), "wz": P["Wz"].astype(bnp),
        "w4": P["W4"].astype(bnp), "w5": P["W5"].astype(bnp),
        "bvec": bvec,
    }
    in_maps = []
    for c in range(N_CORES):
        m = dict(common)
        m["jets"] = _pack_jets(jets, groups, slots[c])
        in_maps.append(m)

    res = run_bass_kernel_spmd(nc, in_maps, core_ids=list(range(N_CORES)),
                               trace=_TRACE[0])
    _LAST_RESULT[0] = res

    agg_x = np.empty((B, 4 * H), np.float32)
    agg_y = np.empty((B, 4 * H), np.float32)
    for c in range(N_CORES):
        ox = np.asarray(res.results[c]["xout"])  # [H, JC] bf16
        oy = np.asarray(res.results[c]["yout"])  # [H, YC] bf16
        joff = 0
        yoff = 0
        for (g, cap), (ids, cnt) in zip(groups, slots[c]):
            PGg = g * (g - 1) // 2
            ii = ids[:cnt]
            xb = ox[:, joff : joff + g * cap].astype(np.float32)
            xb = xb.reshape(H, g, cap)[:, :, :cnt]
            sx = xb.sum(1).T
            mx = xb.max(1).T
            qx = (xb * xb).sum(1).T
            mean_x = sx / g
            agg_x[ii] = np.concatenate(
                [sx, mx, mean_x, qx / g - mean_x * mean_x], axis=1)
            yb = oy[:, yoff : yoff + PGg * cap].astype(np.float32)
            yb = yb.reshape(H, PGg, cap)[:, :, :cnt]
            sy = yb.sum(1).T
            my = yb.max(1).T
            qy = (yb * yb).sum(1).T
            mean_y = sy / PGg
            agg_y[ii] = np.concatenate(
                [sy, my, mean_y, qy / PGg - mean_y * mean_y], axis=1)
            joff += g * cap
            yoff += PGg * cap
    return agg_x, agg_y


# revision 64
# speedup vs baseline: 1.7415x; 1.1392x over previous
# DeepSet Trainium2 kernel.
#
# Strategy: events are sorted by jet-count n (2..10) on the host and
# round-robin sharded across 8 cores into per-group slots of capacity cap_g
# (multiple of 8, exact-packed). Within a group every event has exactly n=g
# valid jets, so all masks, pair structures and aggregation counts are
# compile-time constants.
#
# Math folding (host, O(params)):
#   every Dense+BN+relu block becomes relu(h @ W' + b') with W', b' folded.
#   MLP2 layer 1 uses the z-trick: y1 = relu(z_i + z_j + t) with z = x @ Wz'.
#   t is folded into the y1 relu pass (tensor_scalar add-bias + max0).
#
# Device layout: feature-major [H=128 partitions, columns = slice*cap + b]
# per group, all activations bf16 (PE: 1 col/cycle), PSUM f32.
# The device computes ONLY the MLP chains:
#   jets:  x1 = relu(W1 jt + b1); x2 = relu(W2 x1 + b2); x = relu(W3 x2 + b3)
#          z = Wz x (plain copy evac)
#   pairs: y1 = relu(z_i + z_j + t) (DVE broadcast-add + tensor_scalar 4x)
#          y2 = relu(W4 y1 + b4); y3 = relu(W5 y2 + b5)
# and streams x and y3 (bf16) to DRAM. All aggregations (sum/max/mean/var
# over jets and pairs) happen on the HOST in f32 — the DMA engines were
# ~10% busy while Vector/Scalar were the kernel bottleneck, so shipping
# raw activations beats computing reductions on-device.
#
# Engine split (measured rates, ns/col of 128 rows):
#   Scalar  (~1.05): x1/x2/x/z PSUM evacs, y2 evac, y3 evac (2 of 3)
#   Vector  (~0.65 TT / 0.26 TSP-bf16 / 1.05 PSUM): y1 add, y1 relu,
#           y3 evac (1 of 3)
#   PE:     all matmuls (bf16 1 col/cycle, dual-buffered weight loads)
#   GPSIMD: unused — no PSUM access, no max opcode, slow tensor_scalar,
#           and its tensor_tensor traffic degrades Vector ~25% (measured).
#
# The next group's jets layer-chunks are emitted interleaved into the
# current group's pair-chunk loop (groups in descending size order), so
# PE/Scalar advance the next group while Vector drains the current one.
from contextlib import ExitStack

import numpy as np

import concourse.bass as bass
import concourse.bacc as bacc
import concourse.tile as tile
import concourse.mybir as mybir

f32 = mybir.dt.float32
bf16 = mybir.dt.bfloat16
AF = mybir.ActivationFunctionType
ALU = mybir.AluOpType

H = 128
FJ = 16


def pairs_of(g):
    return [(i, j) for i in range(g) for j in range(i + 1, g)]


# y3 PSUM-evac engine pattern (s=Scalar, v=Vector), tuned from traces.
Y3_PAT = ("s", "v", "v", "s", "v")


def build_program(groups):
    """groups: list of (g, cap) with cap a multiple of 8, cap <= 512."""
    JC = sum(g * cap for g, cap in groups)
    YC = sum((g * (g - 1) // 2) * cap for g, cap in groups)

    nc = bacc.Bacc("TRN2", target_bir_lowering=False, debug=False)

    jets_d = nc.dram_tensor("jets", [FJ, JC], bf16, kind="ExternalInput")
    w1_d = nc.dram_tensor("w1", [FJ, H], bf16, kind="ExternalInput")
    w2_d = nc.dram_tensor("w2", [H, H], bf16, kind="ExternalInput")
    w3_d = nc.dram_tensor("w3", [H, H], bf16, kind="ExternalInput")
    wz_d = nc.dram_tensor("wz", [H, H], bf16, kind="ExternalInput")
    w4_d = nc.dram_tensor("w4", [H, H], bf16, kind="ExternalInput")
    w5_d = nc.dram_tensor("w5", [H, H], bf16, kind="ExternalInput")
    # bias vector cols: 0..5 = b1, b2, b3, t(=bz), b4, b5
    bv_d = nc.dram_tensor("bvec", [H, 8], f32, kind="ExternalInput")
    xout_d = nc.dram_tensor("xout", [H, JC], bf16, kind="ExternalOutput")
    yout_d = nc.dram_tensor("yout", [H, YC], bf16, kind="ExternalOutput")

    with tile.TileContext(nc) as tc, ExitStack() as ctx:
        consts = ctx.enter_context(tc.tile_pool(name="consts", bufs=1))
        big = ctx.enter_context(tc.tile_pool(name="big", bufs=1))
        scr = ctx.enter_context(tc.tile_pool(name="scr", bufs=2))
        mm = ctx.enter_context(tc.tile_pool(name="mm", bufs=4, space="PSUM"))

        def const_tile(name, dram, shape, dt):
            t = consts.tile(shape, dt, tag=name)
            nc.sync.dma_start(t[:], dram.ap())
            return t

        w1t = const_tile("w1", w1_d, [FJ, H], bf16)
        w2t = const_tile("w2", w2_d, [H, H], bf16)
        w3t = const_tile("w3", w3_d, [H, H], bf16)
        wzt = const_tile("wz", wz_d, [H, H], bf16)
        w4t = const_tile("w4", w4_d, [H, H], bf16)
        w5t = const_tile("w5", w5_d, [H, H], bf16)
        bv = const_tile("bv", bv_d, [H, 8], f32)

        def r3(ap, k):
            return ap.rearrange("p (k c) -> p k c", k=k)

        # ---- phase 1: jets MLP over the whole [*, JC] width, no group
        # boundaries. Layer-major: every chunk of a layer only depends on
        # the same chunk of the previous layer (emitted a full layer
        # earlier), so all three engines pipeline freely.
        jt = big.tile([FJ, JC], bf16, tag="jt")
        for c0 in range(0, JC, 2048):
            w = min(2048, JC - c0)
            nc.sync.dma_start(jt[:, c0 : c0 + w],
                              jets_d.ap()[:, c0 : c0 + w])
        x1 = big.tile([H, JC], bf16, tag="x1")
        x2 = big.tile([H, JC], bf16, tag="x2")
        x = big.tile([H, JC], bf16, tag="x")
        z = big.tile([H, JC], bf16, tag="z")
        plan = [(x1, w1t, jt, 0), (x2, w2t, x1, 1),
                (x, w3t, x2, 2), (z, wzt, x, None)]
        for li, (dst, wt, src, bias_col) in enumerate(plan):
            for ci, c0 in enumerate(range(0, JC, 1024)):
                w = min(1024, JC - c0)
                ps = mm.tile([H, 1024], f32, tag="mm")
                for s0 in range(0, w, 512):
                    sw = min(512, w - s0)
                    nc.tensor.matmul(ps[:, s0 : s0 + sw], wt[:],
                                     src[:, c0 + s0 : c0 + s0 + sw],
                                     start=True, stop=True)
                # Alternate the evac engine per chunk within each layer:
                # layers are sequential, so a per-layer split would leave
                # one of Scalar/Vector idle for the whole layer.
                on_v = (ci + li) % 2 == 0
                if bias_col is None:
                    if on_v:
                        nc.vector.tensor_copy(dst[:, c0 : c0 + w],
                                              ps[:, :w])
                    else:
                        nc.scalar.copy(dst[:, c0 : c0 + w], ps[:, :w])
                elif on_v:
                    nc.vector.tensor_scalar(
                        dst[:, c0 : c0 + w], ps[:, :w],
                        bv[:, bias_col : bias_col + 1], 0.0,
                        ALU.add, ALU.max)
                else:
                    nc.scalar.activation(
                        dst[:, c0 : c0 + w], ps[:, :w], AF.Relu,
                        bias=bv[:, bias_col : bias_col + 1])
                if li == 2:
                    nc.sync.dma_start(xout_d.ap()[:, c0 : c0 + w],
                                      dst[:, c0 : c0 + w])

        # ---- phase 2: all pair chunks stream through y1 -> y2 -> y3 ->
        # DRAM. Every chunk's z is ready, so chunks are fully independent
        # and pipeline across V (y1), PE (matmuls), S/V (evacs), DMA.
        # y3 matmuls run one chunk behind y2 (software pipeline) so PE
        # never waits on the y2 evac of the chunk it just produced.
        yout_off = 0
        y3_i = [0]
        pend = [None]  # (y2_tile, w, yout_off)

        def emit_y3(y2, w, yoff):
            y3 = scr.tile([H, 4096], bf16, tag="y3")
            for n0 in range(0, w, 1024):
                cw = min(1024, w - n0)
                ps = mm.tile([H, 1024], f32, tag="mm")
                for s0 in range(0, cw, 512):
                    sw = min(512, cw - s0)
                    nc.tensor.matmul(ps[:, s0 : s0 + sw], w5t[:],
                                     y2[:, n0 + s0 : n0 + s0 + sw],
                                     start=True, stop=True)
                dst = y3[:, n0 : n0 + cw]
                if Y3_PAT[y3_i[0] % len(Y3_PAT)] == "v":
                    nc.vector.tensor_scalar(dst, ps[:, :cw], bv[:, 5:6],
                                            0.0, ALU.add, ALU.max)
                else:
                    nc.scalar.activation(dst, ps[:, :cw], AF.Relu,
                                         bias=bv[:, 5:6])
                y3_i[0] += 1
            nc.sync.dma_start(yout_d.ap()[:, yoff : yoff + w], y3[:, :w])

        jets_off = 0
        for g, cap in groups:
            assert cap % 8 == 0 and cap <= 512
            prs = pairs_of(g)
            PG = len(prs)
            SC = max(1, 4096 // cap)
            for p0 in range(0, PG, SC):
                k = min(SC, PG - p0)
                w = k * cap
                y1 = scr.tile([H, 4096], bf16, tag="y1")
                s = 0
                while s < k:
                    i = prs[p0 + s][0]
                    r = 1
                    while s + r < k and prs[p0 + s + r][0] == i:
                        r += 1
                    j0 = prs[p0 + s][1]
                    zb = jets_off
                    nc.vector.tensor_tensor(
                        r3(y1[:, s * cap : (s + r) * cap], r),
                        r3(z[:, zb + i * cap : zb + (i + 1) * cap],
                           1).broadcast_to([H, r, cap]),
                        r3(z[:, zb + j0 * cap : zb + (j0 + r) * cap], r),
                        ALU.add)
                    s += r
                nc.vector.tensor_scalar(y1[:, :w], y1[:, :w], bv[:, 3:4],
                                        0.0, ALU.add, ALU.max)
                y2 = scr.tile([H, 4096], bf16, tag="y2")
                for n0 in range(0, w, 1024):
                    cw = min(1024, w - n0)
                    ps = mm.tile([H, 1024], f32, tag="mm")
                    for s0 in range(0, cw, 512):
                        sw = min(512, cw - s0)
                        nc.tensor.matmul(ps[:, s0 : s0 + sw], w4t[:],
                                         y1[:, n0 + s0 : n0 + s0 + sw],
                                         start=True, stop=True)
                    nc.scalar.activation(y2[:, n0 : n0 + cw], ps[:, :cw],
                                         AF.Relu, bias=bv[:, 4:5])
                if pend[0] is not None:
                    emit_y3(*pend[0])
                pend[0] = (y2, w, yout_off)
                yout_off += w
            jets_off += g * cap
        emit_y3(*pend[0])

    nc.compile()
    return nc


# ---------------- host-side math ----------------

BN_EPS = 1e-3


def fold_params(inp):
    """Fold normalization + BN into per-layer (W, b). All numpy fp32."""
    mean_j = np.asarray(inp["mean_jets"], np.float32)
    std_j = np.asarray(inp["std_jets"], np.float32)
    w1f = np.asarray(inp["w1_first"], np.float32)
    w1r = np.asarray(inp["w1_rest"], np.float32)
    bn1 = np.asarray(inp["bn1"], np.float32)  # [3,4,H]: gamma, beta, mean, var
    w2f = np.asarray(inp["w2_first"], np.float32)
    w2r = np.asarray(inp["w2_rest"], np.float32)
    bn2 = np.asarray(inp["bn2"], np.float32)

    def bn_sb(row):
        gm, bt, mu, vv = row[0], row[1], row[2], row[3]
        s = gm / np.sqrt(vv + BN_EPS)
        return s.astype(np.float32), (bt - mu * s).astype(np.float32)

    s11, t11 = bn_sb(bn1[0]); s12, t12 = bn_sb(bn1[1]); s13, t13 = bn_sb(bn1[2])
    s21, t21 = bn_sb(bn2[0]); s22, t22 = bn_sb(bn2[1]); s23, t23 = bn_sb(bn2[2])

    A = w1f / std_j[:, None]
    c = -(mean_j / std_j) @ w1f
    return dict(
        W1=A * s11[None, :], b1=c * s11 + t11,
        W2=w1r[0] * s12[None, :], b2=t12,
        W3=w1r[1] * s13[None, :], b3=t13,
        Wz=w2f * s21[None, :], bz=t21,
        W4=w2r[0] * s22[None, :], b4=t22,
        W5=w2r[1] * s23[None, :], b5=t23,
    )


# ---------------- full kernel entry point ----------------

N_CORES = 8

_cache = {}
_TRACE = [False]
_LAST_RESULT = [None]


def _get_program(groups_key):
    if groups_key not in _cache:
        _cache[groups_key] = build_program(list(groups_key))
    return _cache[groups_key]


def _np_dt(dt):
    return mybir.dt.np(dt)


def _plan(n):
    """Returns (groups, slots): groups = [(g, cap)], slots[c][gi] =
    (padded index array, real count) for core c, group gi."""
    gs = []
    idx_by_g = {}
    for g in range(2, 11):
        idx = np.nonzero(n == g)[0]
        if len(idx):
            gs.append(g)
            idx_by_g[g] = idx
    stray = np.nonzero((n < 2) | (n > 10))[0]
    if len(stray):
        if not gs:
            gs.append(2)
            idx_by_g[2] = stray
        else:
            idx_by_g[gs[-1]] = np.concatenate([idx_by_g[gs[-1]], stray])
    # Descending size order: each group's jets chain is emitted inside
    # the previous (bigger) group's pair phase, so it pipelines fully.
    gs = sorted(gs, key=lambda g: -g)
    groups = []
    slots = [[] for _ in range(N_CORES)]
    for g in gs:
        idx = idx_by_g[g]
        per_core = [idx[c::N_CORES] for c in range(N_CORES)]
        mx = max(len(p) for p in per_core)
        cap = max(8, ((mx + 7) // 8) * 8)
        groups.append((g, cap))
        fill = idx[0]
        for c in range(N_CORES):
            p = per_core[c]
            pad = np.full(cap, p[0] if len(p) else fill, dtype=np.int64)
            pad[: len(p)] = p
            slots[c].append((pad, len(p)))
    return groups, slots


def _pack_jets(jets, groups, slots_c):
    cols = []
    for (g, cap), (ids, _cnt) in zip(groups, slots_c):
        ev = jets[ids][:, :g, :]  # [cap, g, 16]
        cols.append(np.ascontiguousarray(ev.transpose(2, 1, 0)).reshape(
            FJ, g * cap))
    return np.concatenate(cols, axis=1).astype(_np_dt(bf16), copy=False)


def kernel(**inputs):
    from concourse.bass_utils import run_bass_kernel_spmd

    jets = np.asarray(inputs["inputs_jets"], dtype=np.float32)
    B = jets.shape[0]
    mask = (jets != 0.0).any(-1)
    n = mask.sum(-1).astype(np.int64)
    # compact valid jets to the front (no-op for the standard generator)
    if not np.array_equal(mask, np.arange(jets.shape[1])[None, :] < n[:, None]):
        order = np.argsort(~mask, axis=1, kind="stable")
        jets = np.take_along_axis(jets, order[:, :, None], axis=1)

    P = fold_params(inputs)
    groups, slots = _plan(n)
    nc = _get_program(tuple(groups))

    bvec = np.zeros((H, 8), np.float32)
    for i, k in enumerate(["b1", "b2", "b3", "bz", "b4", "b5"]):
        bvec[:, i] = P[k]
    bnp = _np_dt(bf16)
    common = {
        "w1": P["W1"].astype(bnp), "w2": P["W2"].astype(bnp),
        "w3": P["W3"].astype(bnp), "wz": P["Wz"].astype(bnp),
        "w4": P["W4"].astype(bnp), "w5": P["W5"].astype(bnp),
        "bvec": bvec,
    }
    in_maps = []
    for c in range(N_CORES):
        m = dict(common)
        m["jets"] = _pack_jets(jets, groups, slots[c])
        in_maps.append(m)

    res = run_bass_kernel_spmd(nc, in_maps, core_ids=list(range(N_CORES)),
                               trace=_TRACE[0])
    _LAST_RESULT[0] = res

    agg_x = np.empty((B, 4 * H), np.float32)
    agg_y = np.empty((B, 4 * H), np.float32)
    for c in range(N_CORES):
        ox = np.asarray(res.results[c]["xout"])  # [H, JC] bf16
        oy = np.asarray(res.results[c]["yout"])  # [H, YC] bf16
        joff = 0
        yoff = 0
        for (g, cap), (ids, cnt) in zip(groups, slots[c]):
            PGg = g * (g - 1) // 2
            ii = ids[:cnt]
            xb = ox[:, joff : joff + g * cap].astype(np.float32)
            xb = xb.reshape(H, g, cap)[:, :, :cnt]
            sx = xb.sum(1).T
            mx = xb.max(1).T
            qx = (xb * xb).sum(1).T
            mean_x = sx / g
            agg_x[ii] = np.concatenate(
                [sx, mx, mean_x, qx / g - mean_x * mean_x], axis=1)
            yb = oy[:, yoff : yoff + PGg * cap].astype(np.float32)
            yb = yb.reshape(H, PGg, cap)[:, :, :cnt]
            sy = yb.sum(1).T
            my = yb.max(1).T
            qy = (yb * yb).sum(1).T
            mean_y = sy / PGg
            agg_y[ii] = np.concatenate(
                [sy, my, mean_y, qy / PGg - mean_y * mean_y], axis=1)
            joff += g * cap
            yoff += PGg * cap
    return agg_x, agg_y


# revision 66
# speedup vs baseline: 1.7501x; 1.0049x over previous
# DeepSet Trainium2 kernel.
#
# Strategy: events are sorted by jet-count n (2..10) on the host and
# round-robin sharded across 8 cores into per-group slots of capacity cap_g
# (multiple of 8, exact-packed). Within a group every event has exactly n=g
# valid jets, so all masks, pair structures and aggregation counts are
# compile-time constants.
#
# Math folding (host, O(params)):
#   every Dense+BN+relu block becomes relu(h @ W' + b') with W', b' folded.
#   MLP2 layer 1 uses the z-trick: y1 = relu(z_i + z_j + t) with z = x @ Wz'.
#   t is folded into the y1 relu pass (tensor_scalar add-bias + max0).
#
# Device layout: feature-major [H=128 partitions, columns = slice*cap + b]
# per group, all activations bf16 (PE: 1 col/cycle), PSUM f32.
# The device computes ONLY the MLP chains:
#   jets:  x1 = relu(W1 jt + b1); x2 = relu(W2 x1 + b2); x = relu(W3 x2 + b3)
#          z = Wz x (plain copy evac)
#   pairs: y1 = relu(z_i + z_j + t) (DVE broadcast-add + tensor_scalar 4x)
#          y2 = relu(W4 y1 + b4); y3 = relu(W5 y2 + b5)
# and streams x and y3 (bf16) to DRAM. All aggregations (sum/max/mean/var
# over jets and pairs) happen on the HOST in f32 — the DMA engines were
# ~10% busy while Vector/Scalar were the kernel bottleneck, so shipping
# raw activations beats computing reductions on-device.
#
# Engine split (measured rates, ns/col of 128 rows):
#   Scalar  (~1.05): x1/x2/x/z PSUM evacs, y2 evac, y3 evac (2 of 3)
#   Vector  (~0.65 TT / 0.26 TSP-bf16 / 1.05 PSUM): y1 add, y1 relu,
#           y3 evac (1 of 3)
#   PE:     all matmuls (bf16 1 col/cycle, dual-buffered weight loads)
#   GPSIMD: unused — no PSUM access, no max opcode, slow tensor_scalar,
#           and its tensor_tensor traffic degrades Vector ~25% (measured).
#
# The next group's jets layer-chunks are emitted interleaved into the
# current group's pair-chunk loop (groups in descending size order), so
# PE/Scalar advance the next group while Vector drains the current one.
from contextlib import ExitStack

import numpy as np

import concourse.bass as bass
import concourse.bacc as bacc
import concourse.tile as tile
import concourse.mybir as mybir

f32 = mybir.dt.float32
bf16 = mybir.dt.bfloat16
AF = mybir.ActivationFunctionType
ALU = mybir.AluOpType

H = 128
FJ = 16


def pairs_of(g):
    return [(i, j) for i in range(g) for j in range(i + 1, g)]


# y3 PSUM-evac engine pattern (s=Scalar, v=Vector), tuned from traces.
Y3_PAT = ("s", "v")


def build_program(groups):
    """groups: list of (g, cap) with cap a multiple of 8, cap <= 512."""
    JC = sum(g * cap for g, cap in groups)
    YC = sum((g * (g - 1) // 2) * cap for g, cap in groups)

    nc = bacc.Bacc("TRN2", target_bir_lowering=False, debug=False)

    jets_d = nc.dram_tensor("jets", [FJ, JC], bf16, kind="ExternalInput")
    w1_d = nc.dram_tensor("w1", [FJ, H], bf16, kind="ExternalInput")
    w2_d = nc.dram_tensor("w2", [H, H], bf16, kind="ExternalInput")
    w3_d = nc.dram_tensor("w3", [H, H], bf16, kind="ExternalInput")
    wz_d = nc.dram_tensor("wz", [H, H], bf16, kind="ExternalInput")
    w4_d = nc.dram_tensor("w4", [H, H], bf16, kind="ExternalInput")
    w5_d = nc.dram_tensor("w5", [H, H], bf16, kind="ExternalInput")
    # bias vector cols: 0..5 = b1, b2, b3, t(=bz), b4, b5
    bv_d = nc.dram_tensor("bvec", [H, 8], f32, kind="ExternalInput")
    xout_d = nc.dram_tensor("xout", [H, JC], bf16, kind="ExternalOutput")
    yout_d = nc.dram_tensor("yout", [H, YC], bf16, kind="ExternalOutput")

    with tile.TileContext(nc) as tc, ExitStack() as ctx:
        consts = ctx.enter_context(tc.tile_pool(name="consts", bufs=1))
        big = ctx.enter_context(tc.tile_pool(name="big", bufs=1))
        scr = ctx.enter_context(tc.tile_pool(name="scr", bufs=2))
        mm = ctx.enter_context(tc.tile_pool(name="mm", bufs=4, space="PSUM"))

        def const_tile(name, dram, shape, dt):
            t = consts.tile(shape, dt, tag=name)
            nc.sync.dma_start(t[:], dram.ap())
            return t

        w1t = const_tile("w1", w1_d, [FJ, H], bf16)
        w2t = const_tile("w2", w2_d, [H, H], bf16)
        w3t = const_tile("w3", w3_d, [H, H], bf16)
        wzt = const_tile("wz", wz_d, [H, H], bf16)
        w4t = const_tile("w4", w4_d, [H, H], bf16)
        w5t = const_tile("w5", w5_d, [H, H], bf16)
        bv = const_tile("bv", bv_d, [H, 8], f32)

        def r3(ap, k):
            return ap.rearrange("p (k c) -> p k c", k=k)

        # ---- phase 1: jets MLP over the whole [*, JC] width, no group
        # boundaries. Layer-major: every chunk of a layer only depends on
        # the same chunk of the previous layer (emitted a full layer
        # earlier), so all three engines pipeline freely.
        jt = big.tile([FJ, JC], bf16, tag="jt")
        for c0 in range(0, JC, 2048):
            w = min(2048, JC - c0)
            nc.sync.dma_start(jt[:, c0 : c0 + w],
                              jets_d.ap()[:, c0 : c0 + w])
        x1 = big.tile([H, JC], bf16, tag="x1")
        x2 = big.tile([H, JC], bf16, tag="x2")
        x = big.tile([H, JC], bf16, tag="x")
        z = big.tile([H, JC], bf16, tag="z")
        plan = [(x1, w1t, jt, 0), (x2, w2t, x1, 1),
                (x, w3t, x2, 2), (z, wzt, x, None)]
        for li, (dst, wt, src, bias_col) in enumerate(plan):
            for ci, c0 in enumerate(range(0, JC, 1024)):
                w = min(1024, JC - c0)
                ps = mm.tile([H, 1024], f32, tag="mm")
                for s0 in range(0, w, 512):
                    sw = min(512, w - s0)
                    nc.tensor.matmul(ps[:, s0 : s0 + sw], wt[:],
                                     src[:, c0 + s0 : c0 + s0 + sw],
                                     start=True, stop=True)
                # Split each layer's evacs between Vector (front half) and
                # Scalar (back half): layers are sequential, so a
                # per-layer split would idle one engine per layer; V gets
                # the front so it finishes early and rolls into y1 (whose
                # first chunks need the first z columns).
                nchunks = -(-JC // 1024)
                on_v = ci < nchunks * 6 // 13
                if bias_col is None:
                    if on_v:
                        nc.vector.tensor_copy(dst[:, c0 : c0 + w],
                                              ps[:, :w])
                    else:
                        nc.scalar.copy(dst[:, c0 : c0 + w], ps[:, :w])
                elif on_v:
                    nc.vector.tensor_scalar(
                        dst[:, c0 : c0 + w], ps[:, :w],
                        bv[:, bias_col : bias_col + 1], 0.0,
                        ALU.add, ALU.max)
                else:
                    nc.scalar.activation(
                        dst[:, c0 : c0 + w], ps[:, :w], AF.Relu,
                        bias=bv[:, bias_col : bias_col + 1])
                if li == 2:
                    nc.sync.dma_start(xout_d.ap()[:, c0 : c0 + w],
                                      dst[:, c0 : c0 + w])

        # ---- phase 2: all pair chunks stream through y1 -> y2 -> y3 ->
        # DRAM. Every chunk's z is ready, so chunks are fully independent
        # and pipeline across V (y1), PE (matmuls), S/V (evacs), DMA.
        # y3 matmuls run one chunk behind y2 (software pipeline) so PE
        # never waits on the y2 evac of the chunk it just produced.
        yout_off = 0
        y3_i = [0]
        pend = [None]  # (y2_tile, w, yout_off)

        def emit_y3(y2, w, yoff):
            y3 = scr.tile([H, 4096], bf16, tag="y3")
            for n0 in range(0, w, 1024):
                cw = min(1024, w - n0)
                ps = mm.tile([H, 1024], f32, tag="mm")
                for s0 in range(0, cw, 512):
                    sw = min(512, cw - s0)
                    nc.tensor.matmul(ps[:, s0 : s0 + sw], w5t[:],
                                     y2[:, n0 + s0 : n0 + s0 + sw],
                                     start=True, stop=True)
                dst = y3[:, n0 : n0 + cw]
                if Y3_PAT[y3_i[0] % len(Y3_PAT)] == "v":
                    nc.vector.tensor_scalar(dst, ps[:, :cw], bv[:, 5:6],
                                            0.0, ALU.add, ALU.max)
                else:
                    nc.scalar.activation(dst, ps[:, :cw], AF.Relu,
                                         bias=bv[:, 5:6])
                y3_i[0] += 1
            nc.sync.dma_start(yout_d.ap()[:, yoff : yoff + w], y3[:, :w])

        jets_off = 0
        for g, cap in groups:
            assert cap % 8 == 0 and cap <= 512
            prs = pairs_of(g)
            PG = len(prs)
            SC = max(1, 4096 // cap)
            for p0 in range(0, PG, SC):
                k = min(SC, PG - p0)
                w = k * cap
                y1 = scr.tile([H, 4096], bf16, tag="y1")
                s = 0
                while s < k:
                    i = prs[p0 + s][0]
                    r = 1
                    while s + r < k and prs[p0 + s + r][0] == i:
                        r += 1
                    j0 = prs[p0 + s][1]
                    zb = jets_off
                    nc.vector.tensor_tensor(
                        r3(y1[:, s * cap : (s + r) * cap], r),
                        r3(z[:, zb + i * cap : zb + (i + 1) * cap],
                           1).broadcast_to([H, r, cap]),
                        r3(z[:, zb + j0 * cap : zb + (j0 + r) * cap], r),
                        ALU.add)
                    s += r
                nc.vector.tensor_scalar(y1[:, :w], y1[:, :w], bv[:, 3:4],
                                        0.0, ALU.add, ALU.max)
                y2 = scr.tile([H, 4096], bf16, tag="y2")
                for n0 in range(0, w, 1024):
                    cw = min(1024, w - n0)
                    ps = mm.tile([H, 1024], f32, tag="mm")
                    for s0 in range(0, cw, 512):
                        sw = min(512, cw - s0)
                        nc.tensor.matmul(ps[:, s0 : s0 + sw], w4t[:],
                                         y1[:, n0 + s0 : n0 + s0 + sw],
                                         start=True, stop=True)
                    nc.scalar.activation(y2[:, n0 : n0 + cw], ps[:, :cw],
                                         AF.Relu, bias=bv[:, 4:5])
                if pend[0] is not None:
                    emit_y3(*pend[0])
                pend[0] = (y2, w, yout_off)
                yout_off += w
            jets_off += g * cap
        emit_y3(*pend[0])

    nc.compile()
    return nc


# ---------------- host-side math ----------------

BN_EPS = 1e-3


def fold_params(inp):
    """Fold normalization + BN into per-layer (W, b). All numpy fp32."""
    mean_j = np.asarray(inp["mean_jets"], np.float32)
    std_j = np.asarray(inp["std_jets"], np.float32)
    w1f = np.asarray(inp["w1_first"], np.float32)
    w1r = np.asarray(inp["w1_rest"], np.float32)
    bn1 = np.asarray(inp["bn1"], np.float32)  # [3,4,H]: gamma, beta, mean, var
    w2f = np.asarray(inp["w2_first"], np.float32)
    w2r = np.asarray(inp["w2_rest"], np.float32)
    bn2 = np.asarray(inp["bn2"], np.float32)

    def bn_sb(row):
        gm, bt, mu, vv = row[0], row[1], row[2], row[3]
        s = gm / np.sqrt(vv + BN_EPS)
        return s.astype(np.float32), (bt - mu * s).astype(np.float32)

    s11, t11 = bn_sb(bn1[0]); s12, t12 = bn_sb(bn1[1]); s13, t13 = bn_sb(bn1[2])
    s21, t21 = bn_sb(bn2[0]); s22, t22 = bn_sb(bn2[1]); s23, t23 = bn_sb(bn2[2])

    A = w1f / std_j[:, None]
    c = -(mean_j / std_j) @ w1f
    return dict(
        W1=A * s11[None, :], b1=c * s11 + t11,
        W2=w1r[0] * s12[None, :], b2=t12,
        W3=w1r[1] * s13[None, :], b3=t13,
        Wz=w2f * s21[None, :], bz=t21,
        W4=w2r[0] * s22[None, :], b4=t22,
        W5=w2r[1] * s23[None, :], b5=t23,
    )


# ---------------- full kernel entry point ----------------

N_CORES = 8

_cache = {}
_TRACE = [False]
_LAST_RESULT = [None]


def _get_program(groups_key):
    if groups_key not in _cache:
        _cache[groups_key] = build_program(list(groups_key))
    return _cache[groups_key]


def _np_dt(dt):
    return mybir.dt.np(dt)


def _plan(n):
    """Returns (groups, slots): groups = [(g, cap)], slots[c][gi] =
    (padded index array, real count) for core c, group gi."""
    gs = []
    idx_by_g = {}
    for g in range(2, 11):
        idx = np.nonzero(n == g)[0]
        if len(idx):
            gs.append(g)
            idx_by_g[g] = idx
    stray = np.nonzero((n < 2) | (n > 10))[0]
    if len(stray):
        if not gs:
            gs.append(2)
            idx_by_g[2] = stray
        else:
            idx_by_g[gs[-1]] = np.concatenate([idx_by_g[gs[-1]], stray])
    # Descending size order: each group's jets chain is emitted inside
    # the previous (bigger) group's pair phase, so it pipelines fully.
    gs = sorted(gs, key=lambda g: -g)
    groups = []
    slots = [[] for _ in range(N_CORES)]
    for g in gs:
        idx = idx_by_g[g]
        per_core = [idx[c::N_CORES] for c in range(N_CORES)]
        mx = max(len(p) for p in per_core)
        cap = max(8, ((mx + 7) // 8) * 8)
        groups.append((g, cap))
        fill = idx[0]
        for c in range(N_CORES):
            p = per_core[c]
            pad = np.full(cap, p[0] if len(p) else fill, dtype=np.int64)
            pad[: len(p)] = p
            slots[c].append((pad, len(p)))
    return groups, slots


def _pack_jets(jets, groups, slots_c):
    cols = []
    for (g, cap), (ids, _cnt) in zip(groups, slots_c):
        ev = jets[ids][:, :g, :]  # [cap, g, 16]
        cols.append(np.ascontiguousarray(ev.transpose(2, 1, 0)).reshape(
            FJ, g * cap))
    return np.concatenate(cols, axis=1).astype(_np_dt(bf16), copy=False)


def kernel(**inputs):
    from concourse.bass_utils import run_bass_kernel_spmd

    jets = np.asarray(inputs["inputs_jets"], dtype=np.float32)
    B = jets.shape[0]
    mask = (jets != 0.0).any(-1)
    n = mask.sum(-1).astype(np.int64)
    # compact valid jets to the front (no-op for the standard generator)
    if not np.array_equal(mask, np.arange(jets.shape[1])[None, :] < n[:, None]):
        order = np.argsort(~mask, axis=1, kind="stable")
        jets = np.take_along_axis(jets, order[:, :, None], axis=1)

    P = fold_params(inputs)
    groups, slots = _plan(n)
    nc = _get_program(tuple(groups))

    bvec = np.zeros((H, 8), np.float32)
    for i, k in enumerate(["b1", "b2", "b3", "bz", "b4", "b5"]):
        bvec[:, i] = P[k]
    bnp = _np_dt(bf16)
    common = {
        "w1": P["W1"].astype(bnp), "w2": P["W2"].astype(bnp),
        "w3": P["W3"].astype(bnp), "wz": P["Wz"].astype(bnp),
        "w4": P["W4"].astype(bnp), "w5": P["W5"].astype(bnp),
        "bvec": bvec,
    }
    in_maps = []
    for c in range(N_CORES):
        m = dict(common)
        m["jets"] = _pack_jets(jets, groups, slots[c])
        in_maps.append(m)

    res = run_bass_kernel_spmd(nc, in_maps, core_ids=list(range(N_CORES)),
                               trace=_TRACE[0])
    _LAST_RESULT[0] = res

    agg_x = np.empty((B, 4 * H), np.float32)
    agg_y = np.empty((B, 4 * H), np.float32)
    for c in range(N_CORES):
        ox = np.asarray(res.results[c]["xout"])  # [H, JC] bf16
        oy = np.asarray(res.results[c]["yout"])  # [H, YC] bf16
        joff = 0
        yoff = 0
        for (g, cap), (ids, cnt) in zip(groups, slots[c]):
            PGg = g * (g - 1) // 2
            ii = ids[:cnt]
            xb = ox[:, joff : joff + g * cap].astype(np.float32)
            xb = xb.reshape(H, g, cap)[:, :, :cnt]
            sx = xb.sum(1).T
            mx = xb.max(1).T
            qx = (xb * xb).sum(1).T
            mean_x = sx / g
            agg_x[ii] = np.concatenate(
                [sx, mx, mean_x, qx / g - mean_x * mean_x], axis=1)
            yb = oy[:, yoff : yoff + PGg * cap].astype(np.float32)
            yb = yb.reshape(H, PGg, cap)[:, :, :cnt]
            sy = yb.sum(1).T
            my = yb.max(1).T
            qy = (yb * yb).sum(1).T
            mean_y = sy / PGg
            agg_y[ii] = np.concatenate(
                [sy, my, mean_y, qy / PGg - mean_y * mean_y], axis=1)
            joff += g * cap
            yoff += PGg * cap
    return agg_x, agg_y


# revision 67
# speedup vs baseline: 1.7783x; 1.0161x over previous
# DeepSet Trainium2 kernel.
#
# Strategy: events are sorted by jet-count n (2..10) on the host and
# round-robin sharded across 8 cores into per-group slots of capacity cap_g
# (multiple of 8, exact-packed). Within a group every event has exactly n=g
# valid jets, so all masks, pair structures and aggregation counts are
# compile-time constants.
#
# Math folding (host, O(params)):
#   every Dense+BN+relu block becomes relu(h @ W' + b') with W', b' folded.
#   MLP2 layer 1 uses the z-trick: y1 = relu(z_i + z_j + t) with z = x @ Wz'.
#   t is folded into the y1 relu pass (tensor_scalar add-bias + max0).
#
# Device layout: feature-major [H=128 partitions, columns = slice*cap + b]
# per group, all activations bf16 (PE: 1 col/cycle), PSUM f32.
# The device computes ONLY the MLP chains:
#   jets:  x1 = relu(W1 jt + b1); x2 = relu(W2 x1 + b2); x = relu(W3 x2 + b3)
#          z = Wz x (plain copy evac)
#   pairs: y1 = relu(z_i + z_j + t) (DVE broadcast-add + tensor_scalar 4x)
#          y2 = relu(W4 y1 + b4); y3 = relu(W5 y2 + b5)
# and streams x and y3 (bf16) to DRAM. All aggregations (sum/max/mean/var
# over jets and pairs) happen on the HOST in f32 — the DMA engines were
# ~10% busy while Vector/Scalar were the kernel bottleneck, so shipping
# raw activations beats computing reductions on-device.
#
# Engine split (measured rates, ns/col of 128 rows):
#   Scalar  (~1.05): x1/x2/x/z PSUM evacs, y2 evac, y3 evac (2 of 3)
#   Vector  (~0.65 TT / 0.26 TSP-bf16 / 1.05 PSUM): y1 add, y1 relu,
#           y3 evac (1 of 3)
#   PE:     all matmuls (bf16 1 col/cycle, dual-buffered weight loads)
#   GPSIMD: unused — no PSUM access, no max opcode, slow tensor_scalar,
#           and its tensor_tensor traffic degrades Vector ~25% (measured).
#
# The next group's jets layer-chunks are emitted interleaved into the
# current group's pair-chunk loop (groups in descending size order), so
# PE/Scalar advance the next group while Vector drains the current one.
from contextlib import ExitStack

import numpy as np

import concourse.bass as bass
import concourse.bacc as bacc
import concourse.tile as tile
import concourse.mybir as mybir

f32 = mybir.dt.float32
bf16 = mybir.dt.bfloat16
AF = mybir.ActivationFunctionType
ALU = mybir.AluOpType

H = 128
FJ = 16


def pairs_of(g):
    return [(i, j) for i in range(g) for j in range(i + 1, g)]


# y3 PSUM-evac engine pattern (s=Scalar, v=Vector), tuned from traces.
Y3_PAT = ("s", "v")


def build_program(groups):
    """groups: list of (g, cap) with cap a multiple of 8, cap <= 512."""
    JC = sum(g * cap for g, cap in groups)
    YC = sum((g * (g - 1) // 2) * cap for g, cap in groups)

    nc = bacc.Bacc("TRN2", target_bir_lowering=False, debug=False)

    jets_d = nc.dram_tensor("jets", [FJ, JC], bf16, kind="ExternalInput")
    w1_d = nc.dram_tensor("w1", [FJ, H], bf16, kind="ExternalInput")
    w2_d = nc.dram_tensor("w2", [H, H], bf16, kind="ExternalInput")
    w3_d = nc.dram_tensor("w3", [H, H], bf16, kind="ExternalInput")
    wz_d = nc.dram_tensor("wz", [H, H], bf16, kind="ExternalInput")
    w4_d = nc.dram_tensor("w4", [H, H], bf16, kind="ExternalInput")
    w5_d = nc.dram_tensor("w5", [H, H], bf16, kind="ExternalInput")
    # bias vector cols: 0..5 = b1, b2, b3, t(=bz), b4, b5
    bv_d = nc.dram_tensor("bvec", [H, 8], f32, kind="ExternalInput")
    xout_d = nc.dram_tensor("xout", [H, JC], bf16, kind="ExternalOutput")
    yout_d = nc.dram_tensor("yout", [H, YC], bf16, kind="ExternalOutput")

    with tile.TileContext(nc) as tc, ExitStack() as ctx:
        consts = ctx.enter_context(tc.tile_pool(name="consts", bufs=1))
        big = ctx.enter_context(tc.tile_pool(name="big", bufs=1))
        scr = ctx.enter_context(tc.tile_pool(name="scr", bufs=3))
        mm = ctx.enter_context(tc.tile_pool(name="mm", bufs=4, space="PSUM"))

        def const_tile(name, dram, shape, dt):
            t = consts.tile(shape, dt, tag=name)
            nc.sync.dma_start(t[:], dram.ap())
            return t

        w1t = const_tile("w1", w1_d, [FJ, H], bf16)
        w2t = const_tile("w2", w2_d, [H, H], bf16)
        w3t = const_tile("w3", w3_d, [H, H], bf16)
        wzt = const_tile("wz", wz_d, [H, H], bf16)
        w4t = const_tile("w4", w4_d, [H, H], bf16)
        w5t = const_tile("w5", w5_d, [H, H], bf16)
        bv = const_tile("bv", bv_d, [H, 8], f32)

        def r3(ap, k):
            return ap.rearrange("p (k c) -> p k c", k=k)

        # ---- phase 1: jets MLP over the whole [*, JC] width, no group
        # boundaries. Layer-major: every chunk of a layer only depends on
        # the same chunk of the previous layer (emitted a full layer
        # earlier), so all three engines pipeline freely.
        jt = big.tile([FJ, JC], bf16, tag="jt")
        for c0 in range(0, JC, 2048):
            w = min(2048, JC - c0)
            nc.sync.dma_start(jt[:, c0 : c0 + w],
                              jets_d.ap()[:, c0 : c0 + w])
        x1 = big.tile([H, JC], bf16, tag="x1")
        x2 = big.tile([H, JC], bf16, tag="x2")
        x = big.tile([H, JC], bf16, tag="x")
        z = big.tile([H, JC], bf16, tag="z")
        plan = [(x1, w1t, jt, 0), (x2, w2t, x1, 1),
                (x, w3t, x2, 2), (z, wzt, x, None)]
        for li, (dst, wt, src, bias_col) in enumerate(plan):
            for ci, c0 in enumerate(range(0, JC, 1024)):
                w = min(1024, JC - c0)
                ps = mm.tile([H, 1024], f32, tag="mm")
                for s0 in range(0, w, 512):
                    sw = min(512, w - s0)
                    nc.tensor.matmul(ps[:, s0 : s0 + sw], wt[:],
                                     src[:, c0 + s0 : c0 + s0 + sw],
                                     start=True, stop=True)
                # Split each layer's evacs between Vector (front half) and
                # Scalar (back half): layers are sequential, so a
                # per-layer split would idle one engine per layer; V gets
                # the front so it finishes early and rolls into y1 (whose
                # first chunks need the first z columns).
                nchunks = -(-JC // 1024)
                on_v = ci < nchunks * 6 // 13
                if bias_col is None:
                    if on_v:
                        nc.vector.tensor_copy(dst[:, c0 : c0 + w],
                                              ps[:, :w])
                    else:
                        nc.scalar.copy(dst[:, c0 : c0 + w], ps[:, :w])
                elif on_v:
                    nc.vector.tensor_scalar(
                        dst[:, c0 : c0 + w], ps[:, :w],
                        bv[:, bias_col : bias_col + 1], 0.0,
                        ALU.add, ALU.max)
                else:
                    nc.scalar.activation(
                        dst[:, c0 : c0 + w], ps[:, :w], AF.Relu,
                        bias=bv[:, bias_col : bias_col + 1])
                if li == 2:
                    nc.sync.dma_start(xout_d.ap()[:, c0 : c0 + w],
                                      dst[:, c0 : c0 + w])

        # ---- phase 2: all pair chunks stream through y1 -> y2 -> y3 ->
        # DRAM. Every chunk's z is ready, so chunks are fully independent
        # and pipeline across V (y1), PE (matmuls), S/V (evacs), DMA.
        # y3 matmuls run one chunk behind y2 (software pipeline) so PE
        # never waits on the y2 evac of the chunk it just produced.
        yout_off = 0
        y3_i = [0]
        pend = [None]  # (y2_tile, w, yout_off)

        def emit_y3(y2, w, yoff):
            y3 = scr.tile([H, 4096], bf16, tag="y3")
            for n0 in range(0, w, 1024):
                cw = min(1024, w - n0)
                ps = mm.tile([H, 1024], f32, tag="mm")
                for s0 in range(0, cw, 512):
                    sw = min(512, cw - s0)
                    nc.tensor.matmul(ps[:, s0 : s0 + sw], w5t[:],
                                     y2[:, n0 + s0 : n0 + s0 + sw],
                                     start=True, stop=True)
                dst = y3[:, n0 : n0 + cw]
                if Y3_PAT[y3_i[0] % len(Y3_PAT)] == "v":
                    nc.vector.tensor_scalar(dst, ps[:, :cw], bv[:, 5:6],
                                            0.0, ALU.add, ALU.max)
                else:
                    nc.scalar.activation(dst, ps[:, :cw], AF.Relu,
                                         bias=bv[:, 5:6])
                y3_i[0] += 1
            nc.sync.dma_start(yout_d.ap()[:, yoff : yoff + w], y3[:, :w])

        jets_off = 0
        for g, cap in groups:
            assert cap % 8 == 0 and cap <= 512
            prs = pairs_of(g)
            PG = len(prs)
            SC = max(1, 4096 // cap)
            for p0 in range(0, PG, SC):
                k = min(SC, PG - p0)
                w = k * cap
                y1 = scr.tile([H, 4096], bf16, tag="y1")
                s = 0
                while s < k:
                    i = prs[p0 + s][0]
                    r = 1
                    while s + r < k and prs[p0 + s + r][0] == i:
                        r += 1
                    j0 = prs[p0 + s][1]
                    zb = jets_off
                    nc.vector.tensor_tensor(
                        r3(y1[:, s * cap : (s + r) * cap], r),
                        r3(z[:, zb + i * cap : zb + (i + 1) * cap],
                           1).broadcast_to([H, r, cap]),
                        r3(z[:, zb + j0 * cap : zb + (j0 + r) * cap], r),
                        ALU.add)
                    s += r
                nc.vector.tensor_scalar(y1[:, :w], y1[:, :w], bv[:, 3:4],
                                        0.0, ALU.add, ALU.max)
                y2 = scr.tile([H, 4096], bf16, tag="y2")
                for n0 in range(0, w, 1024):
                    cw = min(1024, w - n0)
                    ps = mm.tile([H, 1024], f32, tag="mm")
                    for s0 in range(0, cw, 512):
                        sw = min(512, cw - s0)
                        nc.tensor.matmul(ps[:, s0 : s0 + sw], w4t[:],
                                         y1[:, n0 + s0 : n0 + s0 + sw],
                                         start=True, stop=True)
                    nc.scalar.activation(y2[:, n0 : n0 + cw], ps[:, :cw],
                                         AF.Relu, bias=bv[:, 4:5])
                if pend[0] is not None:
                    emit_y3(*pend[0])
                pend[0] = (y2, w, yout_off)
                yout_off += w
            jets_off += g * cap
        emit_y3(*pend[0])

    nc.compile()
    return nc


# ---------------- host-side math ----------------

BN_EPS = 1e-3


def fold_params(inp):
    """Fold normalization + BN into per-layer (W, b). All numpy fp32."""
    mean_j = np.asarray(inp["mean_jets"], np.float32)
    std_j = np.asarray(inp["std_jets"], np.float32)
    w1f = np.asarray(inp["w1_first"], np.float32)
    w1r = np.asarray(inp["w1_rest"], np.float32)
    bn1 = np.asarray(inp["bn1"], np.float32)  # [3,4,H]: gamma, beta, mean, var
    w2f = np.asarray(inp["w2_first"], np.float32)
    w2r = np.asarray(inp["w2_rest"], np.float32)
    bn2 = np.asarray(inp["bn2"], np.float32)

    def bn_sb(row):
        gm, bt, mu, vv = row[0], row[1], row[2], row[3]
        s = gm / np.sqrt(vv + BN_EPS)
        return s.astype(np.float32), (bt - mu * s).astype(np.float32)

    s11, t11 = bn_sb(bn1[0]); s12, t12 = bn_sb(bn1[1]); s13, t13 = bn_sb(bn1[2])
    s21, t21 = bn_sb(bn2[0]); s22, t22 = bn_sb(bn2[1]); s23, t23 = bn_sb(bn2[2])

    A = w1f / std_j[:, None]
    c = -(mean_j / std_j) @ w1f
    return dict(
        W1=A * s11[None, :], b1=c * s11 + t11,
        W2=w1r[0] * s12[None, :], b2=t12,
        W3=w1r[1] * s13[None, :], b3=t13,
        Wz=w2f * s21[None, :], bz=t21,
        W4=w2r[0] * s22[None, :], b4=t22,
        W5=w2r[1] * s23[None, :], b5=t23,
    )


# ---------------- full kernel entry point ----------------

N_CORES = 8

_cache = {}
_TRACE = [False]
_LAST_RESULT = [None]


def _get_program(groups_key):
    if groups_key not in _cache:
        _cache[groups_key] = build_program(list(groups_key))
    return _cache[groups_key]


def _np_dt(dt):
    return mybir.dt.np(dt)


def _plan(n):
    """Returns (groups, slots): groups = [(g, cap)], slots[c][gi] =
    (padded index array, real count) for core c, group gi."""
    gs = []
    idx_by_g = {}
    for g in range(2, 11):
        idx = np.nonzero(n == g)[0]
        if len(idx):
            gs.append(g)
            idx_by_g[g] = idx
    stray = np.nonzero((n < 2) | (n > 10))[0]
    if len(stray):
        if not gs:
            gs.append(2)
            idx_by_g[2] = stray
        else:
            idx_by_g[gs[-1]] = np.concatenate([idx_by_g[gs[-1]], stray])
    # Descending size order: each group's jets chain is emitted inside
    # the previous (bigger) group's pair phase, so it pipelines fully.
    gs = sorted(gs, key=lambda g: -g)
    groups = []
    slots = [[] for _ in range(N_CORES)]
    for g in gs:
        idx = idx_by_g[g]
        per_core = [idx[c::N_CORES] for c in range(N_CORES)]
        mx = max(len(p) for p in per_core)
        cap = max(8, ((mx + 7) // 8) * 8)
        groups.append((g, cap))
        fill = idx[0]
        for c in range(N_CORES):
            p = per_core[c]
            pad = np.full(cap, p[0] if len(p) else fill, dtype=np.int64)
            pad[: len(p)] = p
            slots[c].append((pad, len(p)))
    return groups, slots


def _pack_jets(jets, groups, slots_c):
    cols = []
    for (g, cap), (ids, _cnt) in zip(groups, slots_c):
        ev = jets[ids][:, :g, :]  # [cap, g, 16]
        cols.append(np.ascontiguousarray(ev.transpose(2, 1, 0)).reshape(
            FJ, g * cap))
    return np.concatenate(cols, axis=1).astype(_np_dt(bf16), copy=False)


def kernel(**inputs):
    from concourse.bass_utils import run_bass_kernel_spmd

    jets = np.asarray(inputs["inputs_jets"], dtype=np.float32)
    B = jets.shape[0]
    mask = (jets != 0.0).any(-1)
    n = mask.sum(-1).astype(np.int64)
    # compact valid jets to the front (no-op for the standard generator)
    if not np.array_equal(mask, np.arange(jets.shape[1])[None, :] < n[:, None]):
        order = np.argsort(~mask, axis=1, kind="stable")
        jets = np.take_along_axis(jets, order[:, :, None], axis=1)

    P = fold_params(inputs)
    groups, slots = _plan(n)
    nc = _get_program(tuple(groups))

    bvec = np.zeros((H, 8), np.float32)
    for i, k in enumerate(["b1", "b2", "b3", "bz", "b4", "b5"]):
        bvec[:, i] = P[k]
    bnp = _np_dt(bf16)
    common = {
        "w1": P["W1"].astype(bnp), "w2": P["W2"].astype(bnp),
        "w3": P["W3"].astype(bnp), "wz": P["Wz"].astype(bnp),
        "w4": P["W4"].astype(bnp), "w5": P["W5"].astype(bnp),
        "bvec": bvec,
    }
    in_maps = []
    for c in range(N_CORES):
        m = dict(common)
        m["jets"] = _pack_jets(jets, groups, slots[c])
        in_maps.append(m)

    res = run_bass_kernel_spmd(nc, in_maps, core_ids=list(range(N_CORES)),
                               trace=_TRACE[0])
    _LAST_RESULT[0] = res

    agg_x = np.empty((B, 4 * H), np.float32)
    agg_y = np.empty((B, 4 * H), np.float32)
    for c in range(N_CORES):
        ox = np.asarray(res.results[c]["xout"])  # [H, JC] bf16
        oy = np.asarray(res.results[c]["yout"])  # [H, YC] bf16
        joff = 0
        yoff = 0
        for (g, cap), (ids, cnt) in zip(groups, slots[c]):
            PGg = g * (g - 1) // 2
            ii = ids[:cnt]
            xb = ox[:, joff : joff + g * cap].astype(np.float32)
            xb = xb.reshape(H, g, cap)[:, :, :cnt]
            sx = xb.sum(1).T
            mx = xb.max(1).T
            qx = (xb * xb).sum(1).T
            mean_x = sx / g
            agg_x[ii] = np.concatenate(
                [sx, mx, mean_x, qx / g - mean_x * mean_x], axis=1)
            yb = oy[:, yoff : yoff + PGg * cap].astype(np.float32)
            yb = yb.reshape(H, PGg, cap)[:, :, :cnt]
            sy = yb.sum(1).T
            my = yb.max(1).T
            qy = (yb * yb).sum(1).T
            mean_y = sy / PGg
            agg_y[ii] = np.concatenate(
                [sy, my, mean_y, qy / PGg - mean_y * mean_y], axis=1)
            joff += g * cap
            yoff += PGg * cap
    return agg_x, agg_y
